# revision 14
# baseline (speedup 1.0000x reference)
"""Emformer attention Bass/Tile kernel for 8 Trainium2 NeuronCores.

Strategy: data-parallel over batch (B=16 -> 2 batches per core). Everything on
device is computed in a transposed layout so that no on-device transposes are
needed anywhere:

  qT  [D, Q] = Wq  @ q_in^T        (per head h: qT[h]  = [d=128, Q])
  kT  [D, K] = Wk  @ kv_in^T       (per head h: kT[h]  = [d=128, K])
  v   [K, D] = kv_in^T^T @ Wv^T    (per K-tile: [128, D])
  sT  [K, Q] = kT[h]^T-slices @ qT[h]   (PE lhsT = kT slice, rhs = qT)
  pT  [K, Q] = exp(SCALE*sT) * mask01T  (softmax numerator)
  den [1, Q] = onespad^T @ pT           (softmax denominator)
  aT  [d, Q] = v-slice^T @ pT  accumulated over K-tiles, then * 1/den
  oT  [D, Q] = Wo^T-slices^T @ aT

Masking: the shared attention mask becomes a multiplicative 0/1 bf16 tensor
applied to exp(s) (exp of a real score times zero == contribution of a -1e8
masked score after softmax, exactly).  The per-sample key-padding mask is
folded entirely into DATA: kv_in^T columns beyond klength are zeroed on the
host (so padded v rows are zero -> no PV contribution) and the denominator
matmul's stationary operand is a per-(batch,ktile) 0/1 column (so padded keys
don't count).  No NEG_INF arithmetic and no per-partition exp bias needed.
"""

from contextlib import ExitStack

import numpy as np
import ml_dtypes

import concourse.bass as bass
import concourse.bacc as bacc
import concourse.mybir as mybir
import concourse.tile as tile
from concourse.bass_utils import run_bass_kernel_spmd

BF16 = ml_dtypes.bfloat16

# Problem constants (hardcoded per spec)
D = 1024
H = 8
d = D // H  # 128
T = 1024
R = 32
S = 8
M = 8
B = 16
Q = R + T + S   # 1064
K = M + R + T   # 1064
NCORES = 8
BPC = B // NCORES  # batches per core = 2
SCALE = float(d) ** -0.5
NKT = (K + 127) // 128            # 9 K-tiles (last has 40 live rows)
KT8 = NKT - 1
QCH = [(0, 512), (512, 512), (1024, Q - 1024)]   # query chunks
QCOFF = [0, NKT * 512, 2 * NKT * 512]            # m01r column offset per chunk
M01RW = 2 * NKT * 512 + NKT * (Q - 1024)         # 9576
NM = D // 128                     # 8 row-blocks of the D dimension
PAIRS = [(0, 1), (2, 3), (4, 5), (6, 7)]         # paired K-tiles for wide exps

_BF = mybir.dt.bfloat16
_F32 = mybir.dt.float32


def _build_program(has_bq, has_bk, has_bv, has_bo):
    nc = bacc.Bacc("TRN2", target_bir_lowering=False, debug=False,
                   enable_asserts=True, num_devices=NCORES)

    qinT_d = nc.dram_tensor("qinT", [BPC, NM, 128, Q], _BF, kind="ExternalInput").ap()
    kvinT_d = nc.dram_tensor("kvinT", [BPC, NM, 128, K], _BF, kind="ExternalInput").ap()
    wq_d = nc.dram_tensor("wq", [NM, 128, D], _BF, kind="ExternalInput").ap()
    wk_d = nc.dram_tensor("wk", [NM, 128, D], _BF, kind="ExternalInput").ap()
    wv_d = nc.dram_tensor("wv", [NM, 128, D], _BF, kind="ExternalInput").ap()
    wo_d = nc.dram_tensor("wo", [NM, 128, D], _BF, kind="ExternalInput").ap()
    m01_d = nc.dram_tensor("m01", [128, M01RW], _BF, kind="ExternalInput").ap()
    onespz_d = nc.dram_tensor("onespz", [128, BPC * NKT * 15], _BF, kind="ExternalInput").ap()
    if has_bq:
        bq_d = nc.dram_tensor("bq", [128, NM], _F32, kind="ExternalInput").ap()
    if has_bk:
        bk_d = nc.dram_tensor("bk", [128, NM], _F32, kind="ExternalInput").ap()
    if has_bv:
        bvb_d = nc.dram_tensor("bvb", [128, D], _F32, kind="ExternalInput").ap()
    if has_bo:
        bo_d = nc.dram_tensor("bo", [128, NM], _F32, kind="ExternalInput").ap()
    outT_d = nc.dram_tensor("outT", [BPC, D, Q], _F32, kind="ExternalOutput").ap()

    AF = mybir.ActivationFunctionType

    with tile.TileContext(nc) as tc, ExitStack() as ctx:
        # ---- persistent tiles -------------------------------------------
        sbp = ctx.enter_context(tc.tile_pool(name="persist", bufs=1))
        m01t = sbp.tile([128, M01RW], _BF, name="m01t")
        nc.sync.dma_start(m01t[:], m01_d)
        onespzt = sbp.tile([128, BPC * NKT * 15], _BF, name="onespzt")
        nc.sync.dma_start(onespzt[:], onespz_d)
        if has_bq:
            bqt = sbp.tile([128, NM], _F32, name="bqt")
            nc.sync.dma_start(bqt[:], bq_d)
        if has_bk:
            bkt = sbp.tile([128, NM], _F32, name="bkt")
            nc.sync.dma_start(bkt[:], bk_d)
        if has_bv:
            bvbt = sbp.tile([128, D], _F32, name="bvbt")
            nc.sync.dma_start(bvbt[:], bvb_d)
        if has_bo:
            bot = sbp.tile([128, NM], _F32, name="bot")
            nc.sync.dma_start(bot[:], bo_d)

        qTt = [sbp.tile([128, Q], _BF, name=f"qTt{i}") for i in range(NM)]
        kTt = [sbp.tile([128, Q], _BF, name=f"kTt{i}") for i in range(NM)]
        vt = [sbp.tile([128, D], _BF, name=f"vt{i}") for i in range(NKT)]
        attn = [sbp.tile([128, Q], _BF, name=f"attn{i}") for i in range(NM)]
        den_b = sbp.tile([H, Q], _F32, name="den_b")
        den64 = sbp.tile([64, 133], _F32, name="den64")
        rec64 = sbp.tile([64, 133], _F32, name="rec64")
        rec_b = sbp.tile([H, Q], _F32, name="rec_b")

        # ---- pools -------------------------------------------------------
        inp = ctx.enter_context(tc.tile_pool(name="inp", bufs=NM))
        wpool = ctx.enter_context(tc.tile_pool(name="wpool", bufs=2 * NM))
        ppool = ctx.enter_context(tc.tile_pool(name="ppool", bufs=10))
        ostage = ctx.enter_context(tc.tile_pool(name="ostage", bufs=2))
        dstage = ctx.enter_context(tc.tile_pool(name="dstage", bufs=2))
        rpool = ctx.enter_context(tc.tile_pool(name="rpool", bufs=3))
        bcpool = ctx.enter_context(tc.tile_pool(name="bcpool", bufs=3))
        ps_sp = ctx.enter_context(tc.tile_pool(name="ps_sp", bufs=3, space="PSUM"))
        ps_o = ctx.enter_context(tc.tile_pool(name="ps_o", bufs=1, space="PSUM"))
        ps_d = ctx.enter_context(tc.tile_pool(name="ps_d", bufs=1, space="PSUM"))

        def load_w(dram):
            tiles = []
            for i in range(NM):
                wt = wpool.tile([128, D], _BF, tag="w")
                nc.sync.dma_start(wt[:], dram[i])
                tiles.append(wt)
            return tiles

        def pcopy(dst, ps, bias_tile, m, on_act):
            if bias_tile is not None:
                nc.scalar.activation(dst, ps, AF.Identity,
                                     bias=bias_tile[:, m:m + 1])
            elif on_act:
                nc.scalar.copy(dst, ps)
            else:
                nc.vector.tensor_copy(dst, ps)

        def proj_T(in_tiles, w_tiles, out_tiles, bias_tile, fw, on_act):
            # out[m] [128, fw(<=Q)] = sum_kc w[kc][:, m-block].T @ in[kc][:, chunk]
            # paired 512-chunks accumulate into one [128,1024] psum -> 1 copy
            for m in range(NM):
                for (qo, qw) in QCH[:2] + [(1024, fw - 1024)]:
                    ps = ps_sp.tile([128, qw], _F32, tag="sp")
                    for kc in range(NM):
                        nc.tensor.matmul(
                            ps[:, 0:qw], w_tiles[kc][:, m * 128:(m + 1) * 128],
                            in_tiles[kc][:, qo:qo + qw],
                            start=(kc == 0), stop=(kc == NM - 1))
                    pcopy(out_tiles[m][:, qo:qo + qw], ps[:, 0:qw], bias_tile, m, on_act)

        def load_inp(dram, b):
            ts = []
            for i in range(NM):
                t = inp.tile([128, Q], _BF, tag="in")
                nc.sync.dma_start(t[:], dram[b, i])
                ts.append(t)
            return ts

        def proj_blk(in_tiles, w_tiles, out_tile, m, bias_tile, fw, on_act):
            # one output row-block m of a transposed projection
            for (qo, qw) in QCH[:2] + [(1024, fw - 1024)]:
                ps = ps_sp.tile([128, qw], _F32, tag="sp")
                for kc in range(NM):
                    nc.tensor.matmul(
                        ps[:, 0:qw], w_tiles[kc][:, m * 128:(m + 1) * 128],
                        in_tiles[kc][:, qo:qo + qw],
                        start=(kc == 0), stop=(kc == NM - 1))
                pcopy(out_tile[:, qo:qo + qw], ps[:, 0:qw], bias_tile, m, on_act)

        def vproj_blk(kvin, wv_t, kt):
            pw = min(128, K - kt * 128)
            for half in range(2):
                ps = ps_sp.tile([128, 512], _F32, tag="sp")
                for kc in range(NM):
                    nc.tensor.matmul(
                        ps[0:pw, :], kvin[kc][:, kt * 128:kt * 128 + pw],
                        wv_t[kc][:, half * 512:(half + 1) * 512],
                        start=(kc == 0), stop=(kc == NM - 1))
                nc.vector.tensor_copy(vt[kt][0:pw, half * 512:(half + 1) * 512],
                                      ps[0:pw, :])
                if has_bv:
                    nc.vector.tensor_add(
                        vt[kt][0:pw, half * 512:(half + 1) * 512],
                        vt[kt][0:pw, half * 512:(half + 1) * 512],
                        bvbt[0:pw, half * 512:(half + 1) * 512])

        def scores_exp_mask(b, h, ci, qo, qw):
            probs = {}
            for (k0, k1) in PAIRS:
                ps = ps_sp.tile([128, 2 * qw], _F32, tag="sp")
                for j, kt in enumerate((k0, k1)):
                    nc.tensor.matmul(
                        ps[:, j * qw:(j + 1) * qw],
                        kTt[h][:, kt * 128:(kt + 1) * 128],
                        qTt[h][:, qo:qo + qw], start=True, stop=True)
                pt = ppool.tile([128, 1024], _BF, tag="p")
                nc.scalar.activation(pt[:, 0:2 * qw], ps[:, 0:2 * qw],
                                     AF.Exp, scale=SCALE)
                moff = QCOFF[ci] + k0 * qw
                nc.vector.tensor_mul(pt[:, 0:2 * qw], pt[:, 0:2 * qw],
                                     m01t[:, moff:moff + 2 * qw])
                probs[k0] = (pt, 0)
                probs[k1] = (pt, qw)
            pw = K - KT8 * 128
            ps = ps_sp.tile([128, qw], _F32, tag="sp")
            nc.tensor.matmul(ps[0:pw, 0:qw],
                             kTt[h][:, KT8 * 128:KT8 * 128 + pw],
                             qTt[h][:, qo:qo + qw], start=True, stop=True)
            pt8 = ppool.tile([128, 1024], _BF, tag="p")
            nc.scalar.activation(pt8[0:pw, 0:qw], ps[0:pw, 0:qw],
                                 AF.Exp, scale=SCALE)
            moff = QCOFF[ci] + KT8 * qw
            nc.vector.tensor_mul(pt8[0:pw, 0:qw], pt8[0:pw, 0:qw],
                                 m01t[0:pw, moff:moff + qw])
            probs[KT8] = (pt8, 0)
            return probs

        def pv_den(b, h, probs, qo, qw):
            o_ps = ps_o.tile([128, qw], _F32, tag="o")
            for kt in range(NKT):
                pt, po = probs[kt]
                pw = min(128, K - kt * 128)
                nc.tensor.matmul(
                    o_ps[:, 0:qw], vt[kt][0:pw, h * 128:(h + 1) * 128],
                    pt[0:pw, po:po + qw],
                    start=(kt == 0), stop=(kt == NKT - 1))
            d_ps = ps_d.tile([1, qw], _F32, tag="d")
            zb = (b * NKT) * 15
            for kt in range(NKT):
                pt, po = probs[kt]
                pw = min(128, K - kt * 128)
                zo = zb + kt * 15 + 7
                nc.tensor.matmul(
                    d_ps[:, 0:qw], onespzt[0:pw, zo:zo + 1],
                    pt[0:pw, po:po + qw],
                    start=(kt == 0), stop=(kt == NKT - 1))
            nc.vector.tensor_copy(attn[h][:, qo:qo + qw], o_ps[:, 0:qw])
            dst = dstage.tile([1, qw], _F32, tag="ds")
            nc.vector.tensor_copy(dst[:], d_ps[:, 0:qw])
            nc.sync.dma_start(den_b[h:h + 1, qo:qo + qw], dst[:])

        def head(b, h):
            # qc-pipelined: PV of chunk i-1 issues under exp of chunk i
            pend = None
            for ci, (qo, qw) in enumerate(QCH):
                probs = scores_exp_mask(b, h, ci, qo, qw)
                if pend is not None:
                    pv_den(b, h, pend[0], pend[1], pend[2])
                pend = (probs, qo, qw)
            pv_den(b, h, pend[0], pend[1], pend[2])

        def norm(b):
            nc.sync.dma_start(den64[:], den_b[:])
            nc.vector.reciprocal(rec64[:], den64[:])
            nc.sync.dma_start(rec_b[:], rec64[:])
            for h in range(H):
                rt = rpool.tile([1, Q], _F32, tag="rt")
                nc.sync.dma_start(rt[:], rec_b[h:h + 1, :])
                bc = bcpool.tile([128, Q], _F32, tag="bc")
                nc.gpsimd.partition_broadcast(bc[:], rt[:])
                nc.vector.tensor_mul(attn[h][:], attn[h][:], bc[:])
                wu = ps_o.tile([128, 512], _F32, tag="o")
                nc.tensor.matmul(wu[:, :], qTt[h][:, 0:128],
                                 attn[h][:, 0:512], start=True, stop=True)

        def oproj_blk(b, wo_t, m):
            for (qo, qw) in QCH:
                ps = ps_sp.tile([128, qw], _F32, tag="sp")
                for kc in range(NM):
                    nc.tensor.matmul(
                        ps[:, 0:qw], wo_t[kc][:, m * 128:(m + 1) * 128],
                        attn[kc][:, qo:qo + qw],
                        start=(kc == 0), stop=(kc == NM - 1))
                ot = ostage.tile([128, 512], _F32, tag="os")
                if has_bo:
                    nc.scalar.activation(ot[:, 0:qw], ps[:, 0:qw], AF.Identity,
                                         bias=bot[:, m:m + 1])
                else:
                    nc.scalar.copy(ot[:, 0:qw], ps[:, 0:qw])
                nc.sync.dma_start(outT_d[b, m * 128:(m + 1) * 128, qo:qo + qw],
                                  ot[:, 0:qw])

        # ---- batch 0 projections (serial prologue) -----------------------
        qin0 = load_inp(qinT_d, 0)
        wq_t = load_w(wq_d)
        wk_t = load_w(wk_d)
        for m in range(NM):
            proj_blk(qin0, wq_t, qTt[m], m, bqt if has_bq else None, Q, True)
        kvin0 = load_inp(kvinT_d, 0)
        for m in range(NM):
            proj_blk(kvin0, wk_t, kTt[m], m, bkt if has_bk else None, K, True)
        wv_t = load_w(wv_d)
        for kt in range(NKT):
            vproj_blk(kvin0, wv_t, kt)

        # ---- batch 0 attention; batch 1 qT projection rides along --------
        qin1 = load_inp(qinT_d, 1)
        wq_t1 = load_w(wq_d)
        for h in range(H):
            head(0, h)
            proj_blk(qin1, wq_t1, qTt[h], h, bqt if has_bq else None, Q, True)

        # ---- dense PE tail: batch 1 kT/v projections + batch 0 out-proj --
        kvin1 = load_inp(kvinT_d, 1)
        wk_t1 = load_w(wk_d)
        norm(0)
        for m in range(NM):
            proj_blk(kvin1, wk_t1, kTt[m], m, bkt if has_bk else None, K, True)
        wv_t1 = load_w(wv_d)
        wo_t = load_w(wo_d)
        for kt in range(NKT):
            vproj_blk(kvin1, wv_t1, kt)
        for m in range(NM):
            oproj_blk(0, wo_t, m)

        # ---- batch 1 attention -------------------------------------------
        for h in range(H):
            head(1, h)

        # ---- batch 1 normalize + out-proj (epilogue) ---------------------
        norm(1)
        wo_t1 = load_w(wo_d)
        for m in range(NM):
            oproj_blk(1, wo_t1, m)

    nc.compile()
    return nc


_prog_cache = {}


def _get_program(key):
    if key not in _prog_cache:
        _prog_cache[key] = _build_program(*key)
    return _prog_cache[key]


def kernel(utterance, lengths, right_context, summary, mems, attention_mask,
           Wq, bq, Wkv, bkv, Wo, bo):
    utterance = np.asarray(utterance, np.float32)
    right_context = np.asarray(right_context, np.float32)
    summary = np.asarray(summary, np.float32)
    mems = np.asarray(mems, np.float32)
    lengths = np.asarray(lengths)
    attention_mask = np.asarray(attention_mask)
    Wq = np.asarray(Wq, np.float32)
    Wkv = np.asarray(Wkv, np.float32)
    Wo = np.asarray(Wo, np.float32)
    bq = np.asarray(bq, np.float32)
    bkv = np.asarray(bkv, np.float32)
    bo = np.asarray(bo, np.float32)

    # ---- host-side prep (layouts, masks) ---------------------------------
    q_in = np.concatenate([right_context, utterance, summary], axis=0)   # (Q,B,D)
    kv_in = np.concatenate([mems, right_context, utterance], axis=0)     # (K,B,D)
    qinT = np.ascontiguousarray(q_in.transpose(2, 1, 0)).astype(BF16)    # (D,B,Q)
    kvinT = np.ascontiguousarray(kv_in.transpose(2, 1, 0))               # (D,B,K) f32

    rcbl = Q - int(lengths.max()) - S
    klengths = (lengths.astype(np.int64) + M + rcbl).astype(np.int64)    # (B,)
    # fold key padding into the data: padded kv columns -> 0 (so v rows are 0)
    gk = np.arange(K)
    for bb in range(B):
        kvinT[:, bb, gk >= klengths[bb]] = 0.0
    kvinT = kvinT.astype(BF16)

    wq_h = np.ascontiguousarray(Wq.T).reshape(NM, 128, D).astype(BF16)
    wk_h = np.ascontiguousarray(Wkv[:D].T).reshape(NM, 128, D).astype(BF16)
    wv_h = np.ascontiguousarray(Wkv[D:].T).reshape(NM, 128, D).astype(BF16)
    wo_h = np.ascontiguousarray(Wo.T).reshape(NM, 128, D).astype(BF16)

    m01 = (~attention_mask).T.astype(BF16)                                # (K,Q)
    m01_p = np.zeros((NKT * 128, Q), BF16)
    m01_p[:K] = m01
    # repack into per-(qchunk, ktile) blocks: col = QCOFF[ci] + kt*qw
    m01r = np.zeros((128, M01RW), BF16)
    for ci, (qo, qw) in enumerate(QCH):
        blk = m01_p[:, qo:qo + qw].reshape(NKT, 128, qw).transpose(1, 0, 2)
        m01r[:, QCOFF[ci]:QCOFF[ci] + NKT * qw] = blk.reshape(128, NKT * qw)

    has_bq = bool(np.any(bq))
    has_bk = bool(np.any(bkv[:D]))
    has_bv = bool(np.any(bkv[D:]))
    has_bo = bool(np.any(bo))

    nc = _get_program((has_bq, has_bk, has_bv, has_bo))

    gidx = np.arange(NKT * 128)
    in_maps = []
    for c in range(NCORES):
        bs = [c * BPC + j for j in range(BPC)]
        onespz = np.zeros((128, BPC * NKT, 15), BF16)
        for j, bb in enumerate(bs):
            col = (gidx < klengths[bb]).astype(BF16).reshape(NKT, 128).T
            onespz[:, j * NKT:(j + 1) * NKT, 7] = col
        onespz = onespz.reshape(128, BPC * NKT * 15)
        im = {
            "qinT": np.ascontiguousarray(
                qinT[:, bs, :].transpose(1, 0, 2).reshape(BPC, NM, 128, Q)),
            "kvinT": np.ascontiguousarray(
                kvinT[:, bs, :].transpose(1, 0, 2).reshape(BPC, NM, 128, K)),
            "wq": wq_h, "wk": wk_h, "wv": wv_h, "wo": wo_h,
            "m01": m01r, "onespz": onespz,
        }
        if has_bq:
            im["bq"] = bq.reshape(NM, 128).T.copy()
        if has_bk:
            im["bk"] = bkv[:D].reshape(NM, 128).T.copy()
        if has_bv:
            im["bvb"] = np.broadcast_to(bkv[D:], (128, D)).copy()
        if has_bo:
            im["bo"] = bo.reshape(NM, 128).T.copy()
        in_maps.append(im)

    res = run_bass_kernel_spmd(nc, in_maps, list(range(NCORES)))

    # ---- gather + unshard -------------------------------------------------
    out = np.empty((Q, B, D), np.float32)
    for c in range(NCORES):
        oT = res.results[c]["outT"]                      # (BPC, D, Q)
        for j in range(BPC):
            out[:, c * BPC + j, :] = oT[j].T
    output = out[:Q - S]                                 # (R+T, B, D)
    out_mems = np.clip(out[Q - S:], -10.0, 10.0)[:-1]    # (S-1, B, D)
    return output, out_mems


# revision 15
# speedup vs baseline: 1.0094x; 1.0094x over previous
"""Emformer attention Bass/Tile kernel for 8 Trainium2 NeuronCores.

Strategy: data-parallel over batch (B=16 -> 2 batches per core). Everything on
device is computed in a transposed layout so that no on-device transposes are
needed anywhere:

  qT  [D, Q] = Wq  @ q_in^T        (per head h: qT[h]  = [d=128, Q])
  kT  [D, K] = Wk  @ kv_in^T       (per head h: kT[h]  = [d=128, K])
  v   [K, D] = kv_in^T^T @ Wv^T    (per K-tile: [128, D])
  sT  [K, Q] = kT[h]^T-slices @ qT[h]   (PE lhsT = kT slice, rhs = qT)
  pT  [K, Q] = exp(SCALE*sT) * mask01T  (softmax numerator)
  den [1, Q] = onespad^T @ pT           (softmax denominator)
  aT  [d, Q] = v-slice^T @ pT  accumulated over K-tiles, then * 1/den
  oT  [D, Q] = Wo^T-slices^T @ aT

Masking: the shared attention mask becomes a multiplicative 0/1 bf16 tensor
applied to exp(s) (exp of a real score times zero == contribution of a -1e8
masked score after softmax, exactly).  The per-sample key-padding mask is
folded entirely into DATA: kv_in^T columns beyond klength are zeroed on the
host (so padded v rows are zero -> no PV contribution) and the denominator
matmul's stationary operand is a per-(batch,ktile) 0/1 column (so padded keys
don't count).  No NEG_INF arithmetic and no per-partition exp bias needed.
"""

from contextlib import ExitStack

import numpy as np
import ml_dtypes

import concourse.bass as bass
import concourse.bacc as bacc
import concourse.mybir as mybir
import concourse.tile as tile
from concourse.bass_utils import run_bass_kernel_spmd

BF16 = ml_dtypes.bfloat16

# Problem constants (hardcoded per spec)
D = 1024
H = 8
d = D // H  # 128
T = 1024
R = 32
S = 8
M = 8
B = 16
Q = R + T + S   # 1064
K = M + R + T   # 1064
NCORES = 8
BPC = B // NCORES  # batches per core = 2
SCALE = float(d) ** -0.5
NKT = (K + 127) // 128            # 9 K-tiles (last has 40 live rows)
KT8 = NKT - 1
QCH = [(0, 512), (512, 512), (1024, Q - 1024)]   # query chunks
QCOFF = [0, NKT * 512, 2 * NKT * 512]            # m01r column offset per chunk
M01RW = 2 * NKT * 512 + NKT * (Q - 1024)         # 9576
NM = D // 128                     # 8 row-blocks of the D dimension
PAIRS = [(0, 1), (2, 3), (4, 5), (6, 7)]         # paired K-tiles for wide exps

_BF = mybir.dt.bfloat16
_F32 = mybir.dt.float32


def _build_program(has_bq, has_bk, has_bv, has_bo):
    nc = bacc.Bacc("TRN2", target_bir_lowering=False, debug=False,
                   enable_asserts=True, num_devices=NCORES)

    qinT_d = nc.dram_tensor("qinT", [BPC, NM, 128, Q], _BF, kind="ExternalInput").ap()
    kvinT_d = nc.dram_tensor("kvinT", [BPC, NM, 128, K], _BF, kind="ExternalInput").ap()
    wq_d = nc.dram_tensor("wq", [NM, 128, D], _BF, kind="ExternalInput").ap()
    wk_d = nc.dram_tensor("wk", [NM, 128, D], _BF, kind="ExternalInput").ap()
    wv_d = nc.dram_tensor("wv", [NM, 128, D], _BF, kind="ExternalInput").ap()
    wo_d = nc.dram_tensor("wo", [NM, 128, D], _BF, kind="ExternalInput").ap()
    m01_d = nc.dram_tensor("m01", [128, M01RW], _BF, kind="ExternalInput").ap()
    onespz_d = nc.dram_tensor("onespz", [128, BPC * NKT * 15], _BF, kind="ExternalInput").ap()
    if has_bq:
        bq_d = nc.dram_tensor("bq", [128, NM], _F32, kind="ExternalInput").ap()
    if has_bk:
        bk_d = nc.dram_tensor("bk", [128, NM], _F32, kind="ExternalInput").ap()
    if has_bv:
        bvb_d = nc.dram_tensor("bvb", [128, D], _F32, kind="ExternalInput").ap()
    if has_bo:
        bo_d = nc.dram_tensor("bo", [128, NM], _F32, kind="ExternalInput").ap()
    outT_d = nc.dram_tensor("outT", [BPC, D, Q], _F32, kind="ExternalOutput").ap()

    AF = mybir.ActivationFunctionType

    with tile.TileContext(nc) as tc, ExitStack() as ctx:
        # ---- persistent tiles -------------------------------------------
        sbp = ctx.enter_context(tc.tile_pool(name="persist", bufs=1))
        m01t = sbp.tile([128, M01RW], _BF, name="m01t")
        nc.sync.dma_start(m01t[:], m01_d)
        onespzt = sbp.tile([128, BPC * NKT * 15], _BF, name="onespzt")
        nc.sync.dma_start(onespzt[:], onespz_d)
        if has_bq:
            bqt = sbp.tile([128, NM], _F32, name="bqt")
            nc.sync.dma_start(bqt[:], bq_d)
        if has_bk:
            bkt = sbp.tile([128, NM], _F32, name="bkt")
            nc.sync.dma_start(bkt[:], bk_d)
        if has_bv:
            bvbt = sbp.tile([128, D], _F32, name="bvbt")
            nc.sync.dma_start(bvbt[:], bvb_d)
        if has_bo:
            bot = sbp.tile([128, NM], _F32, name="bot")
            nc.sync.dma_start(bot[:], bo_d)

        qTt = [sbp.tile([128, Q], _BF, name=f"qTt{i}") for i in range(NM)]
        kTt = [sbp.tile([128, Q], _BF, name=f"kTt{i}") for i in range(NM)]
        vt = [sbp.tile([128, D], _BF, name=f"vt{i}") for i in range(NKT)]
        attn = [sbp.tile([128, Q], _BF, name=f"attn{i}") for i in range(NM)]
        den_b = sbp.tile([H, Q], _F32, name="den_b")
        den64 = sbp.tile([64, 133], _F32, name="den64")
        rec64 = sbp.tile([64, 133], _F32, name="rec64")
        rec_b = sbp.tile([H, Q], _F32, name="rec_b")

        # ---- pools -------------------------------------------------------
        inp = ctx.enter_context(tc.tile_pool(name="inp", bufs=NM))
        wpool = ctx.enter_context(tc.tile_pool(name="wpool", bufs=2 * NM))
        ppool = ctx.enter_context(tc.tile_pool(name="ppool", bufs=10))
        ostage = ctx.enter_context(tc.tile_pool(name="ostage", bufs=2))
        dstage = ctx.enter_context(tc.tile_pool(name="dstage", bufs=2))
        rpool = ctx.enter_context(tc.tile_pool(name="rpool", bufs=3))
        bcpool = ctx.enter_context(tc.tile_pool(name="bcpool", bufs=3))
        ps_sp = ctx.enter_context(tc.tile_pool(name="ps_sp", bufs=2, space="PSUM"))
        ps_pp = ctx.enter_context(tc.tile_pool(name="ps_pp", bufs=2, space="PSUM"))
        ps_o = ctx.enter_context(tc.tile_pool(name="ps_o", bufs=1, space="PSUM"))
        ps_d = ctx.enter_context(tc.tile_pool(name="ps_d", bufs=1, space="PSUM"))

        def load_w(dram):
            tiles = []
            for i in range(NM):
                wt = wpool.tile([128, D], _BF, tag="w")
                nc.sync.dma_start(wt[:], dram[i])
                tiles.append(wt)
            return tiles

        def pcopy(dst, ps, bias_tile, m, on_act):
            if bias_tile is not None:
                nc.scalar.activation(dst, ps, AF.Identity,
                                     bias=bias_tile[:, m:m + 1])
            elif on_act:
                nc.scalar.copy(dst, ps)
            else:
                nc.vector.tensor_copy(dst, ps)

        def proj_T(in_tiles, w_tiles, out_tiles, bias_tile, fw, on_act):
            # out[m] [128, fw(<=Q)] = sum_kc w[kc][:, m-block].T @ in[kc][:, chunk]
            # paired 512-chunks accumulate into one [128,1024] psum -> 1 copy
            for m in range(NM):
                for (qo, qw) in QCH[:2] + [(1024, fw - 1024)]:
                    ps = ps_sp.tile([128, qw], _F32, tag="sp")
                    for kc in range(NM):
                        nc.tensor.matmul(
                            ps[:, 0:qw], w_tiles[kc][:, m * 128:(m + 1) * 128],
                            in_tiles[kc][:, qo:qo + qw],
                            start=(kc == 0), stop=(kc == NM - 1))
                    pcopy(out_tiles[m][:, qo:qo + qw], ps[:, 0:qw], bias_tile, m, on_act)

        def load_inp(dram, b):
            ts = []
            for i in range(NM):
                t = inp.tile([128, Q], _BF, tag="in")
                nc.sync.dma_start(t[:], dram[b, i])
                ts.append(t)
            return ts

        def proj_blk(in_tiles, w_tiles, out_tile, m, bias_tile, fw, on_act):
            # one output row-block m of a transposed projection
            for (qo, qw) in QCH[:2] + [(1024, fw - 1024)]:
                ps = ps_pp.tile([128, qw], _F32, tag="pp")
                for kc in range(NM):
                    nc.tensor.matmul(
                        ps[:, 0:qw], w_tiles[kc][:, m * 128:(m + 1) * 128],
                        in_tiles[kc][:, qo:qo + qw],
                        start=(kc == 0), stop=(kc == NM - 1))
                pcopy(out_tile[:, qo:qo + qw], ps[:, 0:qw], bias_tile, m, on_act)

        def vproj_blk(kvin, wv_t, kt):
            pw = min(128, K - kt * 128)
            for half in range(2):
                ps = ps_pp.tile([128, 512], _F32, tag="pp")
                for kc in range(NM):
                    nc.tensor.matmul(
                        ps[0:pw, :], kvin[kc][:, kt * 128:kt * 128 + pw],
                        wv_t[kc][:, half * 512:(half + 1) * 512],
                        start=(kc == 0), stop=(kc == NM - 1))
                nc.vector.tensor_copy(vt[kt][0:pw, half * 512:(half + 1) * 512],
                                      ps[0:pw, :])
                if has_bv:
                    nc.vector.tensor_add(
                        vt[kt][0:pw, half * 512:(half + 1) * 512],
                        vt[kt][0:pw, half * 512:(half + 1) * 512],
                        bvbt[0:pw, half * 512:(half + 1) * 512])

        def scores_exp_mask(b, h, ci, qo, qw):
            probs = {}
            for (k0, k1) in PAIRS:
                ps = ps_sp.tile([128, 2 * qw], _F32, tag="sp")
                for j, kt in enumerate((k0, k1)):
                    nc.tensor.matmul(
                        ps[:, j * qw:(j + 1) * qw],
                        kTt[h][:, kt * 128:(kt + 1) * 128],
                        qTt[h][:, qo:qo + qw], start=True, stop=True)
                pt = ppool.tile([128, 1024], _BF, tag="p")
                nc.scalar.activation(pt[:, 0:2 * qw], ps[:, 0:2 * qw],
                                     AF.Exp, scale=SCALE)
                moff = QCOFF[ci] + k0 * qw
                nc.vector.tensor_mul(pt[:, 0:2 * qw], pt[:, 0:2 * qw],
                                     m01t[:, moff:moff + 2 * qw])
                probs[k0] = (pt, 0)
                probs[k1] = (pt, qw)
            pw = K - KT8 * 128
            ps = ps_sp.tile([128, qw], _F32, tag="sp")
            nc.tensor.matmul(ps[0:pw, 0:qw],
                             kTt[h][:, KT8 * 128:KT8 * 128 + pw],
                             qTt[h][:, qo:qo + qw], start=True, stop=True)
            pt8 = ppool.tile([128, 1024], _BF, tag="p")
            nc.scalar.activation(pt8[0:pw, 0:qw], ps[0:pw, 0:qw],
                                 AF.Exp, scale=SCALE)
            moff = QCOFF[ci] + KT8 * qw
            nc.vector.tensor_mul(pt8[0:pw, 0:qw], pt8[0:pw, 0:qw],
                                 m01t[0:pw, moff:moff + qw])
            probs[KT8] = (pt8, 0)
            return probs

        def pv_den(b, h, probs, qo, qw):
            o_ps = ps_o.tile([128, qw], _F32, tag="o")
            for kt in range(NKT):
                pt, po = probs[kt]
                pw = min(128, K - kt * 128)
                nc.tensor.matmul(
                    o_ps[:, 0:qw], vt[kt][0:pw, h * 128:(h + 1) * 128],
                    pt[0:pw, po:po + qw],
                    start=(kt == 0), stop=(kt == NKT - 1))
            d_ps = ps_d.tile([1, qw], _F32, tag="d")
            zb = (b * NKT) * 15
            for kt in range(NKT):
                pt, po = probs[kt]
                pw = min(128, K - kt * 128)
                zo = zb + kt * 15 + 7
                nc.tensor.matmul(
                    d_ps[:, 0:qw], onespzt[0:pw, zo:zo + 1],
                    pt[0:pw, po:po + qw],
                    start=(kt == 0), stop=(kt == NKT - 1))
            nc.vector.tensor_copy(attn[h][:, qo:qo + qw], o_ps[:, 0:qw])
            dst = dstage.tile([1, qw], _F32, tag="ds")
            nc.vector.tensor_copy(dst[:], d_ps[:, 0:qw])
            nc.sync.dma_start(den_b[h:h + 1, qo:qo + qw], dst[:])

        def head(b, h):
            # qc-pipelined: PV of chunk i-1 issues under exp of chunk i
            pend = None
            for ci, (qo, qw) in enumerate(QCH):
                probs = scores_exp_mask(b, h, ci, qo, qw)
                if pend is not None:
                    pv_den(b, h, pend[0], pend[1], pend[2])
                pend = (probs, qo, qw)
            pv_den(b, h, pend[0], pend[1], pend[2])

        def norm(b):
            nc.sync.dma_start(den64[:], den_b[:])
            nc.vector.reciprocal(rec64[:], den64[:])
            nc.sync.dma_start(rec_b[:], rec64[:])
            for h in range(H):
                rt = rpool.tile([1, Q], _F32, tag="rt")
                nc.sync.dma_start(rt[:], rec_b[h:h + 1, :])
                bc = bcpool.tile([128, Q], _F32, tag="bc")
                nc.gpsimd.partition_broadcast(bc[:], rt[:])
                nc.vector.tensor_mul(attn[h][:], attn[h][:], bc[:])
                wu = ps_o.tile([128, 512], _F32, tag="o")
                nc.tensor.matmul(wu[:, :], qTt[h][:, 0:128],
                                 attn[h][:, 0:512], start=True, stop=True)

        def oproj_blk(b, wo_t, m):
            for (qo, qw) in QCH:
                ps = ps_pp.tile([128, qw], _F32, tag="pp")
                for kc in range(NM):
                    nc.tensor.matmul(
                        ps[:, 0:qw], wo_t[kc][:, m * 128:(m + 1) * 128],
                        attn[kc][:, qo:qo + qw],
                        start=(kc == 0), stop=(kc == NM - 1))
                ot = ostage.tile([128, 512], _F32, tag="os")
                if has_bo:
                    nc.scalar.activation(ot[:, 0:qw], ps[:, 0:qw], AF.Identity,
                                         bias=bot[:, m:m + 1])
                else:
                    nc.scalar.copy(ot[:, 0:qw], ps[:, 0:qw])
                nc.sync.dma_start(outT_d[b, m * 128:(m + 1) * 128, qo:qo + qw],
                                  ot[:, 0:qw])

        # ---- batch 0 projections (serial prologue) -----------------------
        qin0 = load_inp(qinT_d, 0)
        wq_t = load_w(wq_d)
        wk_t = load_w(wk_d)
        for m in range(NM):
            proj_blk(qin0, wq_t, qTt[m], m, bqt if has_bq else None, Q, True)
        kvin0 = load_inp(kvinT_d, 0)
        for m in range(NM):
            proj_blk(kvin0, wk_t, kTt[m], m, bkt if has_bk else None, K, True)
        wv_t = load_w(wv_d)
        for kt in range(NKT):
            vproj_blk(kvin0, wv_t, kt)

        # ---- batch 0 attention; batch 1 qT projection rides along --------
        qin1 = load_inp(qinT_d, 1)
        wq_t1 = load_w(wq_d)
        for h in range(H):
            head(0, h)
            proj_blk(qin1, wq_t1, qTt[h], h, bqt if has_bq else None, Q, True)

        # ---- dense PE tail: batch 1 kT/v projections + batch 0 out-proj --
        kvin1 = load_inp(kvinT_d, 1)
        wk_t1 = load_w(wk_d)
        norm(0)
        for m in range(NM):
            proj_blk(kvin1, wk_t1, kTt[m], m, bkt if has_bk else None, K, True)
        wv_t1 = load_w(wv_d)
        wo_t = load_w(wo_d)
        for kt in range(NKT):
            vproj_blk(kvin1, wv_t1, kt)
        for m in range(NM):
            oproj_blk(0, wo_t, m)

        # ---- batch 1 attention -------------------------------------------
        for h in range(H):
            head(1, h)

        # ---- batch 1 normalize + out-proj (epilogue) ---------------------
        norm(1)
        wo_t1 = load_w(wo_d)
        for m in range(NM):
            oproj_blk(1, wo_t1, m)

    nc.compile()
    return nc


_prog_cache = {}


def _get_program(key):
    if key not in _prog_cache:
        _prog_cache[key] = _build_program(*key)
    return _prog_cache[key]


def kernel(utterance, lengths, right_context, summary, mems, attention_mask,
           Wq, bq, Wkv, bkv, Wo, bo):
    utterance = np.asarray(utterance, np.float32)
    right_context = np.asarray(right_context, np.float32)
    summary = np.asarray(summary, np.float32)
    mems = np.asarray(mems, np.float32)
    lengths = np.asarray(lengths)
    attention_mask = np.asarray(attention_mask)
    Wq = np.asarray(Wq, np.float32)
    Wkv = np.asarray(Wkv, np.float32)
    Wo = np.asarray(Wo, np.float32)
    bq = np.asarray(bq, np.float32)
    bkv = np.asarray(bkv, np.float32)
    bo = np.asarray(bo, np.float32)

    # ---- host-side prep (layouts, masks) ---------------------------------
    q_in = np.concatenate([right_context, utterance, summary], axis=0)   # (Q,B,D)
    kv_in = np.concatenate([mems, right_context, utterance], axis=0)     # (K,B,D)
    qinT = np.ascontiguousarray(q_in.transpose(2, 1, 0)).astype(BF16)    # (D,B,Q)
    kvinT = np.ascontiguousarray(kv_in.transpose(2, 1, 0))               # (D,B,K) f32

    rcbl = Q - int(lengths.max()) - S
    klengths = (lengths.astype(np.int64) + M + rcbl).astype(np.int64)    # (B,)
    # fold key padding into the data: padded kv columns -> 0 (so v rows are 0)
    gk = np.arange(K)
    for bb in range(B):
        kvinT[:, bb, gk >= klengths[bb]] = 0.0
    kvinT = kvinT.astype(BF16)

    wq_h = np.ascontiguousarray(Wq.T).reshape(NM, 128, D).astype(BF16)
    wk_h = np.ascontiguousarray(Wkv[:D].T).reshape(NM, 128, D).astype(BF16)
    wv_h = np.ascontiguousarray(Wkv[D:].T).reshape(NM, 128, D).astype(BF16)
    wo_h = np.ascontiguousarray(Wo.T).reshape(NM, 128, D).astype(BF16)

    m01 = (~attention_mask).T.astype(BF16)                                # (K,Q)
    m01_p = np.zeros((NKT * 128, Q), BF16)
    m01_p[:K] = m01
    # repack into per-(qchunk, ktile) blocks: col = QCOFF[ci] + kt*qw
    m01r = np.zeros((128, M01RW), BF16)
    for ci, (qo, qw) in enumerate(QCH):
        blk = m01_p[:, qo:qo + qw].reshape(NKT, 128, qw).transpose(1, 0, 2)
        m01r[:, QCOFF[ci]:QCOFF[ci] + NKT * qw] = blk.reshape(128, NKT * qw)

    has_bq = bool(np.any(bq))
    has_bk = bool(np.any(bkv[:D]))
    has_bv = bool(np.any(bkv[D:]))
    has_bo = bool(np.any(bo))

    nc = _get_program((has_bq, has_bk, has_bv, has_bo))

    gidx = np.arange(NKT * 128)
    in_maps = []
    for c in range(NCORES):
        bs = [c * BPC + j for j in range(BPC)]
        onespz = np.zeros((128, BPC * NKT, 15), BF16)
        for j, bb in enumerate(bs):
            col = (gidx < klengths[bb]).astype(BF16).reshape(NKT, 128).T
            onespz[:, j * NKT:(j + 1) * NKT, 7] = col
        onespz = onespz.reshape(128, BPC * NKT * 15)
        im = {
            "qinT": np.ascontiguousarray(
                qinT[:, bs, :].transpose(1, 0, 2).reshape(BPC, NM, 128, Q)),
            "kvinT": np.ascontiguousarray(
                kvinT[:, bs, :].transpose(1, 0, 2).reshape(BPC, NM, 128, K)),
            "wq": wq_h, "wk": wk_h, "wv": wv_h, "wo": wo_h,
            "m01": m01r, "onespz": onespz,
        }
        if has_bq:
            im["bq"] = bq.reshape(NM, 128).T.copy()
        if has_bk:
            im["bk"] = bkv[:D].reshape(NM, 128).T.copy()
        if has_bv:
            im["bvb"] = np.broadcast_to(bkv[D:], (128, D)).copy()
        if has_bo:
            im["bo"] = bo.reshape(NM, 128).T.copy()
        in_maps.append(im)

    res = run_bass_kernel_spmd(nc, in_maps, list(range(NCORES)))

    # ---- gather + unshard -------------------------------------------------
    out = np.empty((Q, B, D), np.float32)
    for c in range(NCORES):
        oT = res.results[c]["outT"]                      # (BPC, D, Q)
        for j in range(BPC):
            out[:, c * BPC + j, :] = oT[j].T
    output = out[:Q - S]                                 # (R+T, B, D)
    out_mems = np.clip(out[Q - S:], -10.0, 10.0)[:-1]    # (S-1, B, D)
    return output, out_mems


# revision 16
# speedup vs baseline: 1.0232x; 1.0137x over previous
"""Emformer attention Bass/Tile kernel for 8 Trainium2 NeuronCores.

Strategy: data-parallel over batch (B=16 -> 2 batches per core). Everything on
device is computed in a transposed layout so that no on-device transposes are
needed anywhere:

  qT  [D, Q] = Wq  @ q_in^T        (per head h: qT[h]  = [d=128, Q])
  kT  [D, K] = Wk  @ kv_in^T       (per head h: kT[h]  = [d=128, K])
  v   [K, D] = kv_in^T^T @ Wv^T    (per K-tile: [128, D])
  sT  [K, Q] = kT[h]^T-slices @ qT[h]   (PE lhsT = kT slice, rhs = qT)
  pT  [K, Q] = exp(SCALE*sT) * mask01T  (softmax numerator)
  den [1, Q] = onespad^T @ pT           (softmax denominator)
  aT  [d, Q] = v-slice^T @ pT  accumulated over K-tiles, then * 1/den
  oT  [D, Q] = Wo^T-slices^T @ aT

Masking: the shared attention mask becomes a multiplicative 0/1 bf16 tensor
applied to exp(s) (exp of a real score times zero == contribution of a -1e8
masked score after softmax, exactly).  The per-sample key-padding mask is
folded entirely into DATA: kv_in^T columns beyond klength are zeroed on the
host (so padded v rows are zero -> no PV contribution) and the denominator
matmul's stationary operand is a per-(batch,ktile) 0/1 column (so padded keys
don't count).  No NEG_INF arithmetic and no per-partition exp bias needed.
"""

from contextlib import ExitStack

import numpy as np
import ml_dtypes

import concourse.bass as bass
import concourse.bacc as bacc
import concourse.mybir as mybir
import concourse.tile as tile
from concourse.bass_utils import run_bass_kernel_spmd

BF16 = ml_dtypes.bfloat16

# Problem constants (hardcoded per spec)
D = 1024
H = 8
d = D // H  # 128
T = 1024
R = 32
S = 8
M = 8
B = 16
Q = R + T + S   # 1064
K = M + R + T   # 1064
NCORES = 8
BPC = B // NCORES  # batches per core = 2
SCALE = float(d) ** -0.5
NKT = (K + 127) // 128            # 9 K-tiles (last has 40 live rows)
KT8 = NKT - 1
QCH = [(0, 512), (512, 512), (1024, Q - 1024)]   # query chunks
QCOFF = [0, NKT * 512, 2 * NKT * 512]            # m01r column offset per chunk
M01RW = 2 * NKT * 512 + NKT * (Q - 1024)         # 9576
NM = D // 128                     # 8 row-blocks of the D dimension
PAIRS = [(0, 1), (2, 3), (4, 5), (6, 7)]         # paired K-tiles for wide exps

_BF = mybir.dt.bfloat16
_F32 = mybir.dt.float32


def _build_program(has_bq, has_bk, has_bv, has_bo):
    nc = bacc.Bacc("TRN2", target_bir_lowering=False, debug=False,
                   enable_asserts=True, num_devices=NCORES)

    qinT_d = nc.dram_tensor("qinT", [BPC, NM, 128, Q], _BF, kind="ExternalInput").ap()
    kvinT_d = nc.dram_tensor("kvinT", [BPC, NM, 128, K], _BF, kind="ExternalInput").ap()
    wq_d = nc.dram_tensor("wq", [NM, 128, D], _BF, kind="ExternalInput").ap()
    wk_d = nc.dram_tensor("wk", [NM, 128, D], _BF, kind="ExternalInput").ap()
    wv_d = nc.dram_tensor("wv", [NM, 128, D], _BF, kind="ExternalInput").ap()
    wo_d = nc.dram_tensor("wo", [NM, 128, D], _BF, kind="ExternalInput").ap()
    m01_d = nc.dram_tensor("m01", [128, M01RW], _BF, kind="ExternalInput").ap()
    onespz_d = nc.dram_tensor("onespz", [128, BPC * NKT * 15], _BF, kind="ExternalInput").ap()
    if has_bq:
        bq_d = nc.dram_tensor("bq", [128, NM], _F32, kind="ExternalInput").ap()
    if has_bk:
        bk_d = nc.dram_tensor("bk", [128, NM], _F32, kind="ExternalInput").ap()
    if has_bv:
        bvb_d = nc.dram_tensor("bvb", [128, D], _F32, kind="ExternalInput").ap()
    if has_bo:
        bo_d = nc.dram_tensor("bo", [128, NM], _F32, kind="ExternalInput").ap()
    outT_d = nc.dram_tensor("outT", [BPC, D, Q], _F32, kind="ExternalOutput").ap()

    AF = mybir.ActivationFunctionType

    with tile.TileContext(nc) as tc, ExitStack() as ctx:
        # ---- persistent tiles -------------------------------------------
        sbp = ctx.enter_context(tc.tile_pool(name="persist", bufs=1))
        m01t = sbp.tile([128, M01RW], _BF, name="m01t")
        nc.sync.dma_start(m01t[:], m01_d)
        onespzt = sbp.tile([128, BPC * NKT * 15], _BF, name="onespzt")
        nc.sync.dma_start(onespzt[:], onespz_d)
        if has_bq:
            bqt = sbp.tile([128, NM], _F32, name="bqt")
            nc.sync.dma_start(bqt[:], bq_d)
        if has_bk:
            bkt = sbp.tile([128, NM], _F32, name="bkt")
            nc.sync.dma_start(bkt[:], bk_d)
        if has_bv:
            bvbt = sbp.tile([128, D], _F32, name="bvbt")
            nc.sync.dma_start(bvbt[:], bvb_d)
        if has_bo:
            bot = sbp.tile([128, NM], _F32, name="bot")
            nc.sync.dma_start(bot[:], bo_d)

        qTt = [sbp.tile([128, Q], _BF, name=f"qTt{i}") for i in range(NM)]
        kTt = [sbp.tile([128, Q], _BF, name=f"kTt{i}") for i in range(NM)]
        vt = [sbp.tile([128, D], _BF, name=f"vt{i}") for i in range(NKT)]
        attn = [[sbp.tile([128, Q], _BF, name=f"attn{b}_{i}") for i in range(NM)]
                for b in range(BPC)]
        den_b = sbp.tile([H, Q], _F32, name="den_b")
        den64 = sbp.tile([64, 133], _F32, name="den64")
        rec64 = sbp.tile([64, 133], _F32, name="rec64")
        rec_b = sbp.tile([H, Q], _F32, name="rec_b")

        # ---- pools -------------------------------------------------------
        inp = ctx.enter_context(tc.tile_pool(name="inp", bufs=NM))
        wpool = ctx.enter_context(tc.tile_pool(name="wpool", bufs=2 * NM))
        ppool = ctx.enter_context(tc.tile_pool(name="ppool", bufs=9))
        ostage = ctx.enter_context(tc.tile_pool(name="ostage", bufs=2))
        dstage = ctx.enter_context(tc.tile_pool(name="dstage", bufs=1))
        rpool = ctx.enter_context(tc.tile_pool(name="rpool", bufs=2))
        rbpool = ctx.enter_context(tc.tile_pool(name="rbpool", bufs=2))
        bcpool = ctx.enter_context(tc.tile_pool(name="bcpool", bufs=3))
        ps_sp = ctx.enter_context(tc.tile_pool(name="ps_sp", bufs=2, space="PSUM"))
        ps_pp = ctx.enter_context(tc.tile_pool(name="ps_pp", bufs=2, space="PSUM"))
        ps_o = ctx.enter_context(tc.tile_pool(name="ps_o", bufs=1, space="PSUM"))
        ps_d = ctx.enter_context(tc.tile_pool(name="ps_d", bufs=1, space="PSUM"))

        def load_w(dram):
            tiles = []
            for i in range(NM):
                wt = wpool.tile([128, D], _BF, tag="w")
                nc.sync.dma_start(wt[:], dram[i])
                tiles.append(wt)
            return tiles

        def pcopy(dst, ps, bias_tile, m, on_act):
            if bias_tile is not None:
                nc.scalar.activation(dst, ps, AF.Identity,
                                     bias=bias_tile[:, m:m + 1])
            elif on_act:
                nc.scalar.copy(dst, ps)
            else:
                nc.vector.tensor_copy(dst, ps)

        def proj_T(in_tiles, w_tiles, out_tiles, bias_tile, fw, on_act):
            # out[m] [128, fw(<=Q)] = sum_kc w[kc][:, m-block].T @ in[kc][:, chunk]
            # paired 512-chunks accumulate into one [128,1024] psum -> 1 copy
            for m in range(NM):
                for (qo, qw) in QCH[:2] + [(1024, fw - 1024)]:
                    ps = ps_sp.tile([128, qw], _F32, tag="sp")
                    for kc in range(NM):
                        nc.tensor.matmul(
                            ps[:, 0:qw], w_tiles[kc][:, m * 128:(m + 1) * 128],
                            in_tiles[kc][:, qo:qo + qw],
                            start=(kc == 0), stop=(kc == NM - 1))
                    pcopy(out_tiles[m][:, qo:qo + qw], ps[:, 0:qw], bias_tile, m, on_act)

        def load_inp(dram, b):
            ts = []
            for i in range(NM):
                t = inp.tile([128, Q], _BF, tag="in")
                nc.sync.dma_start(t[:], dram[b, i])
                ts.append(t)
            return ts

        def proj_blk(in_tiles, w_tiles, out_tile, m, bias_tile, fw, on_act):
            # one output row-block m of a transposed projection
            for (qo, qw) in QCH[:2] + [(1024, fw - 1024)]:
                ps = ps_pp.tile([128, qw], _F32, tag="pp")
                for kc in range(NM):
                    nc.tensor.matmul(
                        ps[:, 0:qw], w_tiles[kc][:, m * 128:(m + 1) * 128],
                        in_tiles[kc][:, qo:qo + qw],
                        start=(kc == 0), stop=(kc == NM - 1))
                pcopy(out_tile[:, qo:qo + qw], ps[:, 0:qw], bias_tile, m, on_act)

        def vproj_blk(kvin, wv_t, kt):
            pw = min(128, K - kt * 128)
            for half in range(2):
                ps = ps_pp.tile([128, 512], _F32, tag="pp")
                for kc in range(NM):
                    nc.tensor.matmul(
                        ps[0:pw, :], kvin[kc][:, kt * 128:kt * 128 + pw],
                        wv_t[kc][:, half * 512:(half + 1) * 512],
                        start=(kc == 0), stop=(kc == NM - 1))
                nc.vector.tensor_copy(vt[kt][0:pw, half * 512:(half + 1) * 512],
                                      ps[0:pw, :])
                if has_bv:
                    nc.vector.tensor_add(
                        vt[kt][0:pw, half * 512:(half + 1) * 512],
                        vt[kt][0:pw, half * 512:(half + 1) * 512],
                        bvbt[0:pw, half * 512:(half + 1) * 512])

        def scores_exp_mask(b, h, ci, qo, qw):
            probs = {}
            for (k0, k1) in PAIRS:
                ps = ps_sp.tile([128, 2 * qw], _F32, tag="sp")
                for j, kt in enumerate((k0, k1)):
                    nc.tensor.matmul(
                        ps[:, j * qw:(j + 1) * qw],
                        kTt[h][:, kt * 128:(kt + 1) * 128],
                        qTt[h][:, qo:qo + qw], start=True, stop=True)
                pt = ppool.tile([128, 1024], _BF, tag="p")
                nc.scalar.activation(pt[:, 0:2 * qw], ps[:, 0:2 * qw],
                                     AF.Exp, scale=SCALE)
                moff = QCOFF[ci] + k0 * qw
                nc.vector.tensor_mul(pt[:, 0:2 * qw], pt[:, 0:2 * qw],
                                     m01t[:, moff:moff + 2 * qw])
                probs[k0] = (pt, 0)
                probs[k1] = (pt, qw)
            pw = K - KT8 * 128
            ps = ps_sp.tile([128, qw], _F32, tag="sp")
            nc.tensor.matmul(ps[0:pw, 0:qw],
                             kTt[h][:, KT8 * 128:KT8 * 128 + pw],
                             qTt[h][:, qo:qo + qw], start=True, stop=True)
            pt8 = ppool.tile([128, 1024], _BF, tag="p")
            nc.scalar.activation(pt8[0:pw, 0:qw], ps[0:pw, 0:qw],
                                 AF.Exp, scale=SCALE)
            moff = QCOFF[ci] + KT8 * qw
            nc.vector.tensor_mul(pt8[0:pw, 0:qw], pt8[0:pw, 0:qw],
                                 m01t[0:pw, moff:moff + qw])
            probs[KT8] = (pt8, 0)
            return probs

        def pv_den(b, h, probs, qo, qw):
            o_ps = ps_o.tile([128, qw], _F32, tag="o")
            for kt in range(NKT):
                pt, po = probs[kt]
                pw = min(128, K - kt * 128)
                nc.tensor.matmul(
                    o_ps[:, 0:qw], vt[kt][0:pw, h * 128:(h + 1) * 128],
                    pt[0:pw, po:po + qw],
                    start=(kt == 0), stop=(kt == NKT - 1))
            d_ps = ps_d.tile([1, qw], _F32, tag="d")
            zb = (b * NKT) * 15
            for kt in range(NKT):
                pt, po = probs[kt]
                pw = min(128, K - kt * 128)
                zo = zb + kt * 15 + 7
                nc.tensor.matmul(
                    d_ps[:, 0:qw], onespzt[0:pw, zo:zo + 1],
                    pt[0:pw, po:po + qw],
                    start=(kt == 0), stop=(kt == NKT - 1))
            nc.vector.tensor_copy(attn[b][h][:, qo:qo + qw], o_ps[:, 0:qw])
            dst = dstage.tile([1, qw], _F32, tag="ds")
            nc.vector.tensor_copy(dst[:], d_ps[:, 0:qw])
            nc.sync.dma_start(den_b[h:h + 1, qo:qo + qw], dst[:])

        def head(b, h):
            # qc-pipelined: PV of chunk i-1 issues under exp of chunk i
            pend = None
            for ci, (qo, qw) in enumerate(QCH):
                probs = scores_exp_mask(b, h, ci, qo, qw)
                if pend is not None:
                    pv_den(b, h, pend[0], pend[1], pend[2])
                pend = (probs, qo, qw)
            pv_den(b, h, pend[0], pend[1], pend[2])

        def norm(b):
            nc.sync.dma_start(den64[:], den_b[:])
            nc.vector.reciprocal(rec64[:], den64[:])
            nc.sync.dma_start(rec_b[:], rec64[:])
            for h in range(H):
                rt = rpool.tile([1, Q], _F32, tag="rt")
                nc.sync.dma_start(rt[:], rec_b[h:h + 1, :])
                rtb = rbpool.tile([1, Q], _BF, tag="rtb")
                nc.vector.tensor_copy(rtb[:], rt[:])
                bc = bcpool.tile([128, Q], _BF, tag="bc")
                nc.gpsimd.partition_broadcast(bc[:], rtb[:])
                nc.vector.tensor_mul(attn[b][h][:], attn[b][h][:], bc[:])
                wu = ps_o.tile([128, 512], _F32, tag="o")
                nc.tensor.matmul(wu[:, :], qTt[h][:, 0:128],
                                 attn[b][h][:, 0:512], start=True, stop=True)

        def oproj_blk(b, wo_t, m):
            for (qo, qw) in QCH:
                ps = ps_pp.tile([128, qw], _F32, tag="pp")
                for kc in range(NM):
                    nc.tensor.matmul(
                        ps[:, 0:qw], wo_t[kc][:, m * 128:(m + 1) * 128],
                        attn[b][kc][:, qo:qo + qw],
                        start=(kc == 0), stop=(kc == NM - 1))
                ot = ostage.tile([128, 512], _F32, tag="os")
                if has_bo:
                    nc.scalar.activation(ot[:, 0:qw], ps[:, 0:qw], AF.Identity,
                                         bias=bot[:, m:m + 1])
                else:
                    nc.scalar.copy(ot[:, 0:qw], ps[:, 0:qw])
                nc.sync.dma_start(outT_d[b, m * 128:(m + 1) * 128, qo:qo + qw],
                                  ot[:, 0:qw])

        # ---- batch 0 projections (serial prologue) -----------------------
        qin0 = load_inp(qinT_d, 0)
        wq_t = load_w(wq_d)
        wk_t = load_w(wk_d)
        for m in range(NM):
            proj_blk(qin0, wq_t, qTt[m], m, bqt if has_bq else None, Q, True)
        kvin0 = load_inp(kvinT_d, 0)
        for m in range(NM):
            proj_blk(kvin0, wk_t, kTt[m], m, bkt if has_bk else None, K, True)
        wv_t = load_w(wv_d)
        for kt in range(NKT):
            vproj_blk(kvin0, wv_t, kt)

        # ---- batch 0 attention; batch 1 qT projection rides along --------
        qin1 = load_inp(qinT_d, 1)
        wq_t1 = load_w(wq_d)
        for h in range(H):
            head(0, h)
            proj_blk(qin1, wq_t1, qTt[h], h, bqt if has_bq else None, Q, True)

        # ---- dense PE tail: batch 1 kT/v projections + batch 0 out-proj --
        kvin1 = load_inp(kvinT_d, 1)
        wk_t1 = load_w(wk_d)
        norm(0)
        for m in range(NM):
            proj_blk(kvin1, wk_t1, kTt[m], m, bkt if has_bk else None, K, True)
        wv_t1 = load_w(wv_d)
        wo_t = load_w(wo_d)
        for kt in range(NKT):
            vproj_blk(kvin1, wv_t1, kt)

        # ---- batch 1 attention; batch 0 out-proj rides along -------------
        for h in range(H):
            head(1, h)
            oproj_blk(0, wo_t, h)

        # ---- batch 1 normalize + out-proj (epilogue) ---------------------
        norm(1)
        wo_t1 = load_w(wo_d)
        for m in range(NM):
            oproj_blk(1, wo_t1, m)

    nc.compile()
    return nc


_prog_cache = {}


def _get_program(key):
    if key not in _prog_cache:
        _prog_cache[key] = _build_program(*key)
    return _prog_cache[key]


def kernel(utterance, lengths, right_context, summary, mems, attention_mask,
           Wq, bq, Wkv, bkv, Wo, bo):
    utterance = np.asarray(utterance, np.float32)
    right_context = np.asarray(right_context, np.float32)
    summary = np.asarray(summary, np.float32)
    mems = np.asarray(mems, np.float32)
    lengths = np.asarray(lengths)
    attention_mask = np.asarray(attention_mask)
    Wq = np.asarray(Wq, np.float32)
    Wkv = np.asarray(Wkv, np.float32)
    Wo = np.asarray(Wo, np.float32)
    bq = np.asarray(bq, np.float32)
    bkv = np.asarray(bkv, np.float32)
    bo = np.asarray(bo, np.float32)

    # ---- host-side prep (layouts, masks) ---------------------------------
    q_in = np.concatenate([right_context, utterance, summary], axis=0)   # (Q,B,D)
    kv_in = np.concatenate([mems, right_context, utterance], axis=0)     # (K,B,D)
    qinT = np.ascontiguousarray(q_in.transpose(2, 1, 0)).astype(BF16)    # (D,B,Q)
    kvinT = np.ascontiguousarray(kv_in.transpose(2, 1, 0))               # (D,B,K) f32

    rcbl = Q - int(lengths.max()) - S
    klengths = (lengths.astype(np.int64) + M + rcbl).astype(np.int64)    # (B,)
    # fold key padding into the data: padded kv columns -> 0 (so v rows are 0)
    gk = np.arange(K)
    for bb in range(B):
        kvinT[:, bb, gk >= klengths[bb]] = 0.0
    kvinT = kvinT.astype(BF16)

    wq_h = np.ascontiguousarray(Wq.T).reshape(NM, 128, D).astype(BF16)
    wk_h = np.ascontiguousarray(Wkv[:D].T).reshape(NM, 128, D).astype(BF16)
    wv_h = np.ascontiguousarray(Wkv[D:].T).reshape(NM, 128, D).astype(BF16)
    wo_h = np.ascontiguousarray(Wo.T).reshape(NM, 128, D).astype(BF16)

    m01 = (~attention_mask).T.astype(BF16)                                # (K,Q)
    m01_p = np.zeros((NKT * 128, Q), BF16)
    m01_p[:K] = m01
    # repack into per-(qchunk, ktile) blocks: col = QCOFF[ci] + kt*qw
    m01r = np.zeros((128, M01RW), BF16)
    for ci, (qo, qw) in enumerate(QCH):
        blk = m01_p[:, qo:qo + qw].reshape(NKT, 128, qw).transpose(1, 0, 2)
        m01r[:, QCOFF[ci]:QCOFF[ci] + NKT * qw] = blk.reshape(128, NKT * qw)

    has_bq = bool(np.any(bq))
    has_bk = bool(np.any(bkv[:D]))
    has_bv = bool(np.any(bkv[D:]))
    has_bo = bool(np.any(bo))

    nc = _get_program((has_bq, has_bk, has_bv, has_bo))

    gidx = np.arange(NKT * 128)
    in_maps = []
    for c in range(NCORES):
        bs = [c * BPC + j for j in range(BPC)]
        onespz = np.zeros((128, BPC * NKT, 15), BF16)
        for j, bb in enumerate(bs):
            col = (gidx < klengths[bb]).astype(BF16).reshape(NKT, 128).T
            onespz[:, j * NKT:(j + 1) * NKT, 7] = col
        onespz = onespz.reshape(128, BPC * NKT * 15)
        im = {
            "qinT": np.ascontiguousarray(
                qinT[:, bs, :].transpose(1, 0, 2).reshape(BPC, NM, 128, Q)),
            "kvinT": np.ascontiguousarray(
                kvinT[:, bs, :].transpose(1, 0, 2).reshape(BPC, NM, 128, K)),
            "wq": wq_h, "wk": wk_h, "wv": wv_h, "wo": wo_h,
            "m01": m01r, "onespz": onespz,
        }
        if has_bq:
            im["bq"] = bq.reshape(NM, 128).T.copy()
        if has_bk:
            im["bk"] = bkv[:D].reshape(NM, 128).T.copy()
        if has_bv:
            im["bvb"] = np.broadcast_to(bkv[D:], (128, D)).copy()
        if has_bo:
            im["bo"] = bo.reshape(NM, 128).T.copy()
        in_maps.append(im)

    res = run_bass_kernel_spmd(nc, in_maps, list(range(NCORES)))

    # ---- gather + unshard -------------------------------------------------
    out = np.empty((Q, B, D), np.float32)
    for c in range(NCORES):
        oT = res.results[c]["outT"]                      # (BPC, D, Q)
        for j in range(BPC):
            out[:, c * BPC + j, :] = oT[j].T
    output = out[:Q - S]                                 # (R+T, B, D)
    out_mems = np.clip(out[Q - S:], -10.0, 10.0)[:-1]    # (S-1, B, D)
    return output, out_mems


# revision 18
# speedup vs baseline: 1.0251x; 1.0019x over previous
"""Emformer attention Bass/Tile kernel for 8 Trainium2 NeuronCores.

Strategy: data-parallel over batch (B=16 -> 2 batches per core). Everything on
device is computed in a transposed layout so that no on-device transposes are
needed anywhere:

  qT  [D, Q] = Wq  @ q_in^T        (per head h: qT[h]  = [d=128, Q])
  kT  [D, K] = Wk  @ kv_in^T       (per head h: kT[h]  = [d=128, K])
  v   [K, D] = kv_in^T^T @ Wv^T    (per K-tile: [128, D])
  sT  [K, Q] = kT[h]^T-slices @ qT[h]   (PE lhsT = kT slice, rhs = qT)
  pT  [K, Q] = exp(SCALE*sT) * mask01T  (softmax numerator)
  den [1, Q] = onespad^T @ pT           (softmax denominator)
  aT  [d, Q] = v-slice^T @ pT  accumulated over K-tiles, then * 1/den
  oT  [D, Q] = Wo^T-slices^T @ aT

Masking: the shared attention mask becomes a multiplicative 0/1 bf16 tensor
applied to exp(s) (exp of a real score times zero == contribution of a -1e8
masked score after softmax, exactly).  The per-sample key-padding mask is
folded entirely into DATA: kv_in^T columns beyond klength are zeroed on the
host (so padded v rows are zero -> no PV contribution) and the denominator
matmul's stationary operand is a per-(batch,ktile) 0/1 column (so padded keys
don't count).  No NEG_INF arithmetic and no per-partition exp bias needed.
"""

from contextlib import ExitStack

import numpy as np
import ml_dtypes

import concourse.bass as bass
import concourse.bacc as bacc
import concourse.mybir as mybir
import concourse.tile as tile
from concourse.bass_utils import run_bass_kernel_spmd

BF16 = ml_dtypes.bfloat16

# Problem constants (hardcoded per spec)
D = 1024
H = 8
d = D // H  # 128
T = 1024
R = 32
S = 8
M = 8
B = 16
Q = R + T + S   # 1064
K = M + R + T   # 1064
NCORES = 8
BPC = B // NCORES  # batches per core = 2
SCALE = float(d) ** -0.5
NKT = (K + 127) // 128            # 9 K-tiles (last has 40 live rows)
KT8 = NKT - 1
QCH = [(0, 512), (512, 512), (1024, Q - 1024)]   # query chunks
QCOFF = [0, NKT * 512, 2 * NKT * 512]            # m01r column offset per chunk
M01RW = 2 * NKT * 512 + NKT * (Q - 1024)         # 9576
NM = D // 128                     # 8 row-blocks of the D dimension
PAIRS = [(0, 1), (2, 3), (4, 5), (6, 7)]         # paired K-tiles for wide exps

_BF = mybir.dt.bfloat16
_F32 = mybir.dt.float32


def _build_program(has_bq, has_bk, has_bv, has_bo):
    nc = bacc.Bacc("TRN2", target_bir_lowering=False, debug=False,
                   enable_asserts=True, num_devices=NCORES)

    qinT_d = nc.dram_tensor("qinT", [BPC, NM, 128, Q], _BF, kind="ExternalInput").ap()
    kvinT_d = nc.dram_tensor("kvinT", [BPC, NM, 128, K], _BF, kind="ExternalInput").ap()
    wq_d = nc.dram_tensor("wq", [NM, 128, D], _BF, kind="ExternalInput").ap()
    wk_d = nc.dram_tensor("wk", [NM, 128, D], _BF, kind="ExternalInput").ap()
    wv_d = nc.dram_tensor("wv", [NM, 128, D], _BF, kind="ExternalInput").ap()
    wo_d = nc.dram_tensor("wo", [NM, 128, D], _BF, kind="ExternalInput").ap()
    m01_d = nc.dram_tensor("m01", [128, M01RW], _BF, kind="ExternalInput").ap()
    onespz_d = nc.dram_tensor("onespz", [128, BPC * NKT * 15], _BF, kind="ExternalInput").ap()
    if has_bq:
        bq_d = nc.dram_tensor("bq", [128, NM], _F32, kind="ExternalInput").ap()
    if has_bk:
        bk_d = nc.dram_tensor("bk", [128, NM], _F32, kind="ExternalInput").ap()
    if has_bv:
        bvb_d = nc.dram_tensor("bvb", [128, D], _F32, kind="ExternalInput").ap()
    if has_bo:
        bo_d = nc.dram_tensor("bo", [128, NM], _F32, kind="ExternalInput").ap()
    outT_d = nc.dram_tensor("outT", [BPC, D, Q], _F32, kind="ExternalOutput").ap()

    AF = mybir.ActivationFunctionType

    with tile.TileContext(nc) as tc, ExitStack() as ctx:
        # ---- persistent tiles -------------------------------------------
        sbp = ctx.enter_context(tc.tile_pool(name="persist", bufs=1))
        m01t = sbp.tile([128, M01RW], _BF, name="m01t")
        nc.sync.dma_start(m01t[:], m01_d)
        onespzt = sbp.tile([128, BPC * NKT * 15], _BF, name="onespzt")
        nc.sync.dma_start(onespzt[:], onespz_d)
        if has_bq:
            bqt = sbp.tile([128, NM], _F32, name="bqt")
            nc.sync.dma_start(bqt[:], bq_d)
        if has_bk:
            bkt = sbp.tile([128, NM], _F32, name="bkt")
            nc.sync.dma_start(bkt[:], bk_d)
        if has_bv:
            bvbt = sbp.tile([128, D], _F32, name="bvbt")
            nc.sync.dma_start(bvbt[:], bvb_d)
        if has_bo:
            bot = sbp.tile([128, NM], _F32, name="bot")
            nc.sync.dma_start(bot[:], bo_d)

        qTt = [sbp.tile([128, Q], _BF, name=f"qTt{i}") for i in range(NM)]
        kTt = [sbp.tile([128, Q], _BF, name=f"kTt{i}") for i in range(NM)]
        vt = [sbp.tile([128, D], _BF, name=f"vt{i}") for i in range(NKT)]
        attn = [[sbp.tile([128, Q], _BF, name=f"attn{b}_{i}") for i in range(NM)]
                for b in range(BPC)]
        den_b = sbp.tile([H, Q], _F32, name="den_b")
        den64 = sbp.tile([64, 133], _F32, name="den64")
        rec64 = sbp.tile([64, 133], _F32, name="rec64")
        rec_b = sbp.tile([H, Q], _F32, name="rec_b")

        # ---- pools -------------------------------------------------------
        inp = ctx.enter_context(tc.tile_pool(name="inp", bufs=NM))
        wpool = ctx.enter_context(tc.tile_pool(name="wpool", bufs=2 * NM))
        ppool = ctx.enter_context(tc.tile_pool(name="ppool", bufs=9))
        ostage = ctx.enter_context(tc.tile_pool(name="ostage", bufs=2))
        dstage = ctx.enter_context(tc.tile_pool(name="dstage", bufs=1))
        rpool = ctx.enter_context(tc.tile_pool(name="rpool", bufs=2))
        rbpool = ctx.enter_context(tc.tile_pool(name="rbpool", bufs=2))
        bcpool = ctx.enter_context(tc.tile_pool(name="bcpool", bufs=3))
        ps_sp = ctx.enter_context(tc.tile_pool(name="ps_sp", bufs=2, space="PSUM"))
        ps_pp = ctx.enter_context(tc.tile_pool(name="ps_pp", bufs=2, space="PSUM"))
        ps_o = ctx.enter_context(tc.tile_pool(name="ps_o", bufs=1, space="PSUM"))
        ps_d = ctx.enter_context(tc.tile_pool(name="ps_d", bufs=1, space="PSUM"))

        def load_w(dram):
            tiles = []
            for i in range(NM):
                wt = wpool.tile([128, D], _BF, tag="w")
                nc.sync.dma_start(wt[:], dram[i])
                tiles.append(wt)
            return tiles

        def pcopy(dst, ps, bias_tile, m, on_act):
            if bias_tile is not None:
                nc.scalar.activation(dst, ps, AF.Identity,
                                     bias=bias_tile[:, m:m + 1])
            elif on_act:
                nc.scalar.copy(dst, ps)
            else:
                nc.vector.tensor_copy(dst, ps)

        def proj_T(in_tiles, w_tiles, out_tiles, bias_tile, fw, on_act):
            # out[m] [128, fw(<=Q)] = sum_kc w[kc][:, m-block].T @ in[kc][:, chunk]
            # paired 512-chunks accumulate into one [128,1024] psum -> 1 copy
            for m in range(NM):
                for (qo, qw) in QCH[:2] + [(1024, fw - 1024)]:
                    ps = ps_sp.tile([128, qw], _F32, tag="sp")
                    for kc in range(NM):
                        nc.tensor.matmul(
                            ps[:, 0:qw], w_tiles[kc][:, m * 128:(m + 1) * 128],
                            in_tiles[kc][:, qo:qo + qw],
                            start=(kc == 0), stop=(kc == NM - 1))
                    pcopy(out_tiles[m][:, qo:qo + qw], ps[:, 0:qw], bias_tile, m, on_act)

        def load_inp(dram, b):
            ts = []
            for i in range(NM):
                t = inp.tile([128, Q], _BF, tag="in")
                nc.sync.dma_start(t[:], dram[b, i])
                ts.append(t)
            return ts

        def proj_blk(in_tiles, w_tiles, out_tile, m, bias_tile, fw, on_act):
            # one output row-block m of a transposed projection
            for (qo, qw) in QCH[:2] + [(1024, fw - 1024)]:
                ps = ps_pp.tile([128, qw], _F32, tag="pp")
                for kc in range(NM):
                    nc.tensor.matmul(
                        ps[:, 0:qw], w_tiles[kc][:, m * 128:(m + 1) * 128],
                        in_tiles[kc][:, qo:qo + qw],
                        start=(kc == 0), stop=(kc == NM - 1))
                pcopy(out_tile[:, qo:qo + qw], ps[:, 0:qw], bias_tile, m, on_act)

        def vproj_blk(kvin, wv_t, kt):
            pw = min(128, K - kt * 128)
            for half in range(2):
                ps = ps_pp.tile([128, 512], _F32, tag="pp")
                for kc in range(NM):
                    nc.tensor.matmul(
                        ps[0:pw, :], kvin[kc][:, kt * 128:kt * 128 + pw],
                        wv_t[kc][:, half * 512:(half + 1) * 512],
                        start=(kc == 0), stop=(kc == NM - 1))
                nc.vector.tensor_copy(vt[kt][0:pw, half * 512:(half + 1) * 512],
                                      ps[0:pw, :])
                if has_bv:
                    nc.vector.tensor_add(
                        vt[kt][0:pw, half * 512:(half + 1) * 512],
                        vt[kt][0:pw, half * 512:(half + 1) * 512],
                        bvbt[0:pw, half * 512:(half + 1) * 512])

        def scores_exp_mask(b, h, ci, qo, qw):
            probs = {}
            for (k0, k1) in PAIRS:
                ps = ps_sp.tile([128, 2 * qw], _F32, tag="sp")
                for j, kt in enumerate((k0, k1)):
                    nc.tensor.matmul(
                        ps[:, j * qw:(j + 1) * qw],
                        kTt[h][:, kt * 128:(kt + 1) * 128],
                        qTt[h][:, qo:qo + qw], start=True, stop=True)
                pt = ppool.tile([128, 1024], _BF, tag="p")
                nc.scalar.activation(pt[:, 0:2 * qw], ps[:, 0:2 * qw],
                                     AF.Exp, scale=SCALE)
                moff = QCOFF[ci] + k0 * qw
                nc.vector.tensor_mul(pt[:, 0:2 * qw], pt[:, 0:2 * qw],
                                     m01t[:, moff:moff + 2 * qw])
                probs[k0] = (pt, 0)
                probs[k1] = (pt, qw)
            pw = K - KT8 * 128
            ps = ps_sp.tile([128, qw], _F32, tag="sp")
            nc.tensor.matmul(ps[0:pw, 0:qw],
                             kTt[h][:, KT8 * 128:KT8 * 128 + pw],
                             qTt[h][:, qo:qo + qw], start=True, stop=True)
            pt8 = ppool.tile([128, 1024], _BF, tag="p")
            nc.scalar.activation(pt8[0:pw, 0:qw], ps[0:pw, 0:qw],
                                 AF.Exp, scale=SCALE)
            moff = QCOFF[ci] + KT8 * qw
            nc.vector.tensor_mul(pt8[0:pw, 0:qw], pt8[0:pw, 0:qw],
                                 m01t[0:pw, moff:moff + qw])
            probs[KT8] = (pt8, 0)
            return probs

        def pv_den(b, h, probs, qo, qw):
            o_ps = ps_o.tile([128, qw], _F32, tag="o")
            for kt in range(NKT):
                pt, po = probs[kt]
                pw = min(128, K - kt * 128)
                nc.tensor.matmul(
                    o_ps[:, 0:qw], vt[kt][0:pw, h * 128:(h + 1) * 128],
                    pt[0:pw, po:po + qw],
                    start=(kt == 0), stop=(kt == NKT - 1))
            d_ps = ps_d.tile([1, qw], _F32, tag="d")
            zb = (b * NKT) * 15
            for kt in range(NKT):
                pt, po = probs[kt]
                pw = min(128, K - kt * 128)
                zo = zb + kt * 15 + 7
                nc.tensor.matmul(
                    d_ps[:, 0:qw], onespzt[0:pw, zo:zo + 1],
                    pt[0:pw, po:po + qw],
                    start=(kt == 0), stop=(kt == NKT - 1))
            nc.vector.tensor_copy(attn[b][h][:, qo:qo + qw], o_ps[:, 0:qw])
            dst = dstage.tile([1, qw], _F32, tag="ds")
            nc.vector.tensor_copy(dst[:], d_ps[:, 0:qw])
            nc.sync.dma_start(den_b[h:h + 1, qo:qo + qw], dst[:])

        def head(b, h):
            # qc-pipelined: PV of chunk i-1 issues under exp of chunk i
            pend = None
            for ci, (qo, qw) in enumerate(QCH):
                probs = scores_exp_mask(b, h, ci, qo, qw)
                if pend is not None:
                    pv_den(b, h, pend[0], pend[1], pend[2])
                pend = (probs, qo, qw)
            pv_den(b, h, pend[0], pend[1], pend[2])

        def norm(b):
            nc.sync.dma_start(den64[:], den_b[:])
            nc.vector.reciprocal(rec64[:], den64[:])
            nc.sync.dma_start(rec_b[:], rec64[:])
            for h in range(H):
                rt = rpool.tile([1, Q], _F32, tag="rt")
                nc.sync.dma_start(rt[:], rec_b[h:h + 1, :])
                rtb = rbpool.tile([1, Q], _BF, tag="rtb")
                nc.vector.tensor_copy(rtb[:], rt[:])
                bc = bcpool.tile([128, Q], _BF, tag="bc")
                nc.gpsimd.partition_broadcast(bc[:], rtb[:])
                wu = ps_o.tile([128, 512], _F32, tag="o")
                nc.tensor.matmul(wu[:, 0:512], qTt[h][:, 0:128],
                                 bc[:, 0:512], start=True, stop=True)
                nc.vector.tensor_mul(attn[b][h][:], attn[b][h][:], bc[:])
                wu2 = ps_o.tile([128, 512], _F32, tag="o")
                nc.tensor.matmul(wu2[:, :], qTt[h][:, 0:128],
                                 attn[b][h][:, 0:512], start=True, stop=True)

        def oproj_blk(b, wo_t, m):
            for (qo, qw) in QCH:
                ps = ps_pp.tile([128, qw], _F32, tag="pp")
                for kc in range(NM):
                    nc.tensor.matmul(
                        ps[:, 0:qw], wo_t[kc][:, m * 128:(m + 1) * 128],
                        attn[b][kc][:, qo:qo + qw],
                        start=(kc == 0), stop=(kc == NM - 1))
                ot = ostage.tile([128, 512], _F32, tag="os")
                if has_bo:
                    nc.scalar.activation(ot[:, 0:qw], ps[:, 0:qw], AF.Identity,
                                         bias=bot[:, m:m + 1])
                else:
                    nc.scalar.copy(ot[:, 0:qw], ps[:, 0:qw])
                nc.sync.dma_start(outT_d[b, m * 128:(m + 1) * 128, qo:qo + qw],
                                  ot[:, 0:qw])

        # ---- batch 0 projections (serial prologue) -----------------------
        qin0 = load_inp(qinT_d, 0)
        wq_t = load_w(wq_d)
        wk_t = load_w(wk_d)
        for m in range(NM):
            proj_blk(qin0, wq_t, qTt[m], m, bqt if has_bq else None, Q, True)
        kvin0 = load_inp(kvinT_d, 0)
        for m in range(NM):
            proj_blk(kvin0, wk_t, kTt[m], m, bkt if has_bk else None, K, True)
        wv_t = load_w(wv_d)
        for kt in range(NKT):
            vproj_blk(kvin0, wv_t, kt)

        # ---- batch 0 attention; batch 1 qT projection rides along --------
        qin1 = load_inp(qinT_d, 1)
        wq_t1 = load_w(wq_d)
        for h in range(H):
            head(0, h)
            proj_blk(qin1, wq_t1, qTt[h], h, bqt if has_bq else None, Q, True)

        # ---- dense PE tail: batch 1 kT/v projections + batch 0 out-proj --
        kvin1 = load_inp(kvinT_d, 1)
        wk_t1 = load_w(wk_d)
        norm(0)
        for m in range(NM):
            proj_blk(kvin1, wk_t1, kTt[m], m, bkt if has_bk else None, K, True)
        wv_t1 = load_w(wv_d)
        wo_t = load_w(wo_d)
        for kt in range(NKT):
            vproj_blk(kvin1, wv_t1, kt)

        # ---- batch 1 attention; batch 0 out-proj rides along -------------
        for h in range(H):
            head(1, h)
            oproj_blk(0, wo_t, h)

        # ---- batch 1 normalize + out-proj (epilogue) ---------------------
        norm(1)
        wo_t1 = load_w(wo_d)
        for m in range(NM):
            oproj_blk(1, wo_t1, m)

    nc.compile()
    return nc


_prog_cache = {}


def _get_program(key):
    if key not in _prog_cache:
        _prog_cache[key] = _build_program(*key)
    return _prog_cache[key]


def kernel(utterance, lengths, right_context, summary, mems, attention_mask,
           Wq, bq, Wkv, bkv, Wo, bo):
    utterance = np.asarray(utterance, np.float32)
    right_context = np.asarray(right_context, np.float32)
    summary = np.asarray(summary, np.float32)
    mems = np.asarray(mems, np.float32)
    lengths = np.asarray(lengths)
    attention_mask = np.asarray(attention_mask)
    Wq = np.asarray(Wq, np.float32)
    Wkv = np.asarray(Wkv, np.float32)
    Wo = np.asarray(Wo, np.float32)
    bq = np.asarray(bq, np.float32)
    bkv = np.asarray(bkv, np.float32)
    bo = np.asarray(bo, np.float32)

    # ---- host-side prep (layouts, masks) ---------------------------------
    q_in = np.concatenate([right_context, utterance, summary], axis=0)   # (Q,B,D)
    kv_in = np.concatenate([mems, right_context, utterance], axis=0)     # (K,B,D)
    qinT = np.ascontiguousarray(q_in.transpose(2, 1, 0)).astype(BF16)    # (D,B,Q)
    kvinT = np.ascontiguousarray(kv_in.transpose(2, 1, 0))               # (D,B,K) f32

    rcbl = Q - int(lengths.max()) - S
    klengths = (lengths.astype(np.int64) + M + rcbl).astype(np.int64)    # (B,)
    # fold key padding into the data: padded kv columns -> 0 (so v rows are 0)
    gk = np.arange(K)
    for bb in range(B):
        kvinT[:, bb, gk >= klengths[bb]] = 0.0
    kvinT = kvinT.astype(BF16)

    wq_h = np.ascontiguousarray(Wq.T).reshape(NM, 128, D).astype(BF16)
    wk_h = np.ascontiguousarray(Wkv[:D].T).reshape(NM, 128, D).astype(BF16)
    wv_h = np.ascontiguousarray(Wkv[D:].T).reshape(NM, 128, D).astype(BF16)
    wo_h = np.ascontiguousarray(Wo.T).reshape(NM, 128, D).astype(BF16)

    m01 = (~attention_mask).T.astype(BF16)                                # (K,Q)
    m01_p = np.zeros((NKT * 128, Q), BF16)
    m01_p[:K] = m01
    # repack into per-(qchunk, ktile) blocks: col = QCOFF[ci] + kt*qw
    m01r = np.zeros((128, M01RW), BF16)
    for ci, (qo, qw) in enumerate(QCH):
        blk = m01_p[:, qo:qo + qw].reshape(NKT, 128, qw).transpose(1, 0, 2)
        m01r[:, QCOFF[ci]:QCOFF[ci] + NKT * qw] = blk.reshape(128, NKT * qw)

    has_bq = bool(np.any(bq))
    has_bk = bool(np.any(bkv[:D]))
    has_bv = bool(np.any(bkv[D:]))
    has_bo = bool(np.any(bo))

    nc = _get_program((has_bq, has_bk, has_bv, has_bo))

    gidx = np.arange(NKT * 128)
    in_maps = []
    for c in range(NCORES):
        bs = [c * BPC + j for j in range(BPC)]
        onespz = np.zeros((128, BPC * NKT, 15), BF16)
        for j, bb in enumerate(bs):
            col = (gidx < klengths[bb]).astype(BF16).reshape(NKT, 128).T
            onespz[:, j * NKT:(j + 1) * NKT, 7] = col
        onespz = onespz.reshape(128, BPC * NKT * 15)
        im = {
            "qinT": np.ascontiguousarray(
                qinT[:, bs, :].transpose(1, 0, 2).reshape(BPC, NM, 128, Q)),
            "kvinT": np.ascontiguousarray(
                kvinT[:, bs, :].transpose(1, 0, 2).reshape(BPC, NM, 128, K)),
            "wq": wq_h, "wk": wk_h, "wv": wv_h, "wo": wo_h,
            "m01": m01r, "onespz": onespz,
        }
        if has_bq:
            im["bq"] = bq.reshape(NM, 128).T.copy()
        if has_bk:
            im["bk"] = bkv[:D].reshape(NM, 128).T.copy()
        if has_bv:
            im["bvb"] = np.broadcast_to(bkv[D:], (128, D)).copy()
        if has_bo:
            im["bo"] = bo.reshape(NM, 128).T.copy()
        in_maps.append(im)

    res = run_bass_kernel_spmd(nc, in_maps, list(range(NCORES)))

    # ---- gather + unshard -------------------------------------------------
    out = np.empty((Q, B, D), np.float32)
    for c in range(NCORES):
        oT = res.results[c]["outT"]                      # (BPC, D, Q)
        for j in range(BPC):
            out[:, c * BPC + j, :] = oT[j].T
    output = out[:Q - S]                                 # (R+T, B, D)
    out_mems = np.clip(out[Q - S:], -10.0, 10.0)[:-1]    # (S-1, B, D)
    return output, out_mems


# revision 19
# speedup vs baseline: 1.0345x; 1.0091x over previous
"""Emformer attention Bass/Tile kernel for 8 Trainium2 NeuronCores.

Strategy: data-parallel over batch (B=16 -> 2 batches per core). Everything on
device is computed in a transposed layout so that no on-device transposes are
needed anywhere:

  qT  [D, Q] = Wq  @ q_in^T        (per head h: qT[h]  = [d=128, Q])
  kT  [D, K] = Wk  @ kv_in^T       (per head h: kT[h]  = [d=128, K])
  v   [K, D] = kv_in^T^T @ Wv^T    (per K-tile: [128, D])
  sT  [K, Q] = kT[h]^T-slices @ qT[h]   (PE lhsT = kT slice, rhs = qT)
  pT  [K, Q] = exp(SCALE*sT) * mask01T  (softmax numerator)
  den [1, Q] = onespad^T @ pT           (softmax denominator)
  aT  [d, Q] = v-slice^T @ pT  accumulated over K-tiles, then * 1/den
  oT  [D, Q] = Wo^T-slices^T @ aT

Masking: the shared attention mask becomes a multiplicative 0/1 bf16 tensor
applied to exp(s) (exp of a real score times zero == contribution of a -1e8
masked score after softmax, exactly).  The per-sample key-padding mask is
folded entirely into DATA: kv_in^T columns beyond klength are zeroed on the
host (so padded v rows are zero -> no PV contribution) and the denominator
matmul's stationary operand is a per-(batch,ktile) 0/1 column (so padded keys
don't count).  No NEG_INF arithmetic and no per-partition exp bias needed.
"""

from contextlib import ExitStack

import numpy as np
import ml_dtypes

import concourse.bass as bass
import concourse.bacc as bacc
import concourse.mybir as mybir
import concourse.tile as tile
from concourse.bass_utils import run_bass_kernel_spmd

BF16 = ml_dtypes.bfloat16

# Problem constants (hardcoded per spec)
D = 1024
H = 8
d = D // H  # 128
T = 1024
R = 32
S = 8
M = 8
B = 16
Q = R + T + S   # 1064
K = M + R + T   # 1064
NCORES = 8
BPC = B // NCORES  # batches per core = 2
SCALE = float(d) ** -0.5
NKT = (K + 127) // 128            # 9 K-tiles (last has 40 live rows)
KT8 = NKT - 1
QCH = [(0, 512), (512, 512), (1024, Q - 1024)]   # query chunks
QCOFF = [0, NKT * 512, 2 * NKT * 512]            # m01r column offset per chunk
M01RW = 2 * NKT * 512 + NKT * (Q - 1024)         # 9576
NM = D // 128                     # 8 row-blocks of the D dimension
PAIRS = [(0, 1), (2, 3), (4, 5), (6, 7)]         # paired K-tiles for wide exps

_BF = mybir.dt.bfloat16
_F32 = mybir.dt.float32


def _build_program(has_bq, has_bk, has_bv, has_bo):
    nc = bacc.Bacc("TRN2", target_bir_lowering=False, debug=False,
                   enable_asserts=True, num_devices=NCORES)

    qinT_d = nc.dram_tensor("qinT", [BPC, NM, 128, Q], _BF, kind="ExternalInput").ap()
    kvinT_d = nc.dram_tensor("kvinT", [BPC, NM, 128, K], _BF, kind="ExternalInput").ap()
    wq_d = nc.dram_tensor("wq", [NM, 128, D], _BF, kind="ExternalInput").ap()
    wk_d = nc.dram_tensor("wk", [NM, 128, D], _BF, kind="ExternalInput").ap()
    wv_d = nc.dram_tensor("wv", [NM, 128, D], _BF, kind="ExternalInput").ap()
    wo_d = nc.dram_tensor("wo", [NM, 128, D], _BF, kind="ExternalInput").ap()
    m01_d = nc.dram_tensor("m01", [128, M01RW], _BF, kind="ExternalInput").ap()
    onespz_d = nc.dram_tensor("onespz", [128, BPC * NKT * 15], _BF, kind="ExternalInput").ap()
    if has_bq:
        bq_d = nc.dram_tensor("bq", [128, NM], _F32, kind="ExternalInput").ap()
    if has_bk:
        bk_d = nc.dram_tensor("bk", [128, NM], _F32, kind="ExternalInput").ap()
    if has_bv:
        bvb_d = nc.dram_tensor("bvb", [128, D], _F32, kind="ExternalInput").ap()
    if has_bo:
        bo_d = nc.dram_tensor("bo", [128, NM], _F32, kind="ExternalInput").ap()
    outT_d = nc.dram_tensor("outT", [BPC, D, Q], _F32, kind="ExternalOutput").ap()

    AF = mybir.ActivationFunctionType

    with tile.TileContext(nc) as tc, ExitStack() as ctx:
        # ---- persistent tiles -------------------------------------------
        sbp = ctx.enter_context(tc.tile_pool(name="persist", bufs=1))
        m01t = sbp.tile([128, M01RW], _BF, name="m01t")
        onespzt = sbp.tile([128, BPC * NKT * 15], _BF, name="onespzt")
        if has_bq:
            bqt = sbp.tile([128, NM], _F32, name="bqt")
            nc.sync.dma_start(bqt[:], bq_d)
        if has_bk:
            bkt = sbp.tile([128, NM], _F32, name="bkt")
            nc.sync.dma_start(bkt[:], bk_d)
        if has_bv:
            bvbt = sbp.tile([128, D], _F32, name="bvbt")
            nc.sync.dma_start(bvbt[:], bvb_d)
        if has_bo:
            bot = sbp.tile([128, NM], _F32, name="bot")
            nc.sync.dma_start(bot[:], bo_d)

        qTt = [sbp.tile([128, Q], _BF, name=f"qTt{i}") for i in range(NM)]
        kTt = [sbp.tile([128, Q], _BF, name=f"kTt{i}") for i in range(NM)]
        vt = [sbp.tile([128, D], _BF, name=f"vt{i}") for i in range(NKT)]
        attn = [[sbp.tile([128, Q], _BF, name=f"attn{b}_{i}") for i in range(NM)]
                for b in range(BPC)]
        den_b = sbp.tile([H, Q], _F32, name="den_b")
        den64 = sbp.tile([64, 133], _F32, name="den64")
        rec64 = sbp.tile([64, 133], _F32, name="rec64")
        rec_b = sbp.tile([H, Q], _F32, name="rec_b")

        # ---- pools -------------------------------------------------------
        inp = ctx.enter_context(tc.tile_pool(name="inp", bufs=NM))
        wpool = ctx.enter_context(tc.tile_pool(name="wpool", bufs=2 * NM))
        ppool = ctx.enter_context(tc.tile_pool(name="ppool", bufs=9))
        ostage = ctx.enter_context(tc.tile_pool(name="ostage", bufs=2))
        dstage = ctx.enter_context(tc.tile_pool(name="dstage", bufs=1))
        rpool = ctx.enter_context(tc.tile_pool(name="rpool", bufs=2))
        rbpool = ctx.enter_context(tc.tile_pool(name="rbpool", bufs=2))
        bcpool = ctx.enter_context(tc.tile_pool(name="bcpool", bufs=3))
        ps_sp = ctx.enter_context(tc.tile_pool(name="ps_sp", bufs=2, space="PSUM"))
        ps_pp = ctx.enter_context(tc.tile_pool(name="ps_pp", bufs=2, space="PSUM"))
        ps_o = ctx.enter_context(tc.tile_pool(name="ps_o", bufs=1, space="PSUM"))
        ps_d = ctx.enter_context(tc.tile_pool(name="ps_d", bufs=1, space="PSUM"))

        def load_w(dram):
            tiles = []
            for i in range(NM):
                wt = wpool.tile([128, D], _BF, tag="w")
                nc.sync.dma_start(wt[:], dram[i])
                tiles.append(wt)
            return tiles

        def pcopy(dst, ps, bias_tile, m, on_act):
            if bias_tile is not None:
                nc.scalar.activation(dst, ps, AF.Identity,
                                     bias=bias_tile[:, m:m + 1])
            elif on_act:
                nc.scalar.copy(dst, ps)
            else:
                nc.vector.tensor_copy(dst, ps)

        def proj_T(in_tiles, w_tiles, out_tiles, bias_tile, fw, on_act):
            # out[m] [128, fw(<=Q)] = sum_kc w[kc][:, m-block].T @ in[kc][:, chunk]
            # paired 512-chunks accumulate into one [128,1024] psum -> 1 copy
            for m in range(NM):
                for (qo, qw) in QCH[:2] + [(1024, fw - 1024)]:
                    ps = ps_sp.tile([128, qw], _F32, tag="sp")
                    for kc in range(NM):
                        nc.tensor.matmul(
                            ps[:, 0:qw], w_tiles[kc][:, m * 128:(m + 1) * 128],
                            in_tiles[kc][:, qo:qo + qw],
                            start=(kc == 0), stop=(kc == NM - 1))
                    pcopy(out_tiles[m][:, qo:qo + qw], ps[:, 0:qw], bias_tile, m, on_act)

        def load_inp(dram, b):
            ts = []
            for i in range(NM):
                t = inp.tile([128, Q], _BF, tag="in")
                nc.sync.dma_start(t[:], dram[b, i])
                ts.append(t)
            return ts

        def proj_blk(in_tiles, w_tiles, out_tile, m, bias_tile, fw, on_act):
            # one output row-block m of a transposed projection
            for (qo, qw) in QCH[:2] + [(1024, fw - 1024)]:
                ps = ps_pp.tile([128, qw], _F32, tag="pp")
                for kc in range(NM):
                    nc.tensor.matmul(
                        ps[:, 0:qw], w_tiles[kc][:, m * 128:(m + 1) * 128],
                        in_tiles[kc][:, qo:qo + qw],
                        start=(kc == 0), stop=(kc == NM - 1))
                pcopy(out_tile[:, qo:qo + qw], ps[:, 0:qw], bias_tile, m, on_act)

        def vproj_blk(kvin, wv_t, kt):
            pw = min(128, K - kt * 128)
            for half in range(2):
                ps = ps_pp.tile([128, 512], _F32, tag="pp")
                for kc in range(NM):
                    nc.tensor.matmul(
                        ps[0:pw, :], kvin[kc][:, kt * 128:kt * 128 + pw],
                        wv_t[kc][:, half * 512:(half + 1) * 512],
                        start=(kc == 0), stop=(kc == NM - 1))
                nc.vector.tensor_copy(vt[kt][0:pw, half * 512:(half + 1) * 512],
                                      ps[0:pw, :])
                if has_bv:
                    nc.vector.tensor_add(
                        vt[kt][0:pw, half * 512:(half + 1) * 512],
                        vt[kt][0:pw, half * 512:(half + 1) * 512],
                        bvbt[0:pw, half * 512:(half + 1) * 512])

        def scores_exp_mask(b, h, ci, qo, qw):
            probs = {}
            for (k0, k1) in PAIRS:
                ps = ps_sp.tile([128, 2 * qw], _F32, tag="sp")
                for j, kt in enumerate((k0, k1)):
                    nc.tensor.matmul(
                        ps[:, j * qw:(j + 1) * qw],
                        kTt[h][:, kt * 128:(kt + 1) * 128],
                        qTt[h][:, qo:qo + qw], start=True, stop=True)
                pt = ppool.tile([128, 1024], _BF, tag="p")
                nc.scalar.activation(pt[:, 0:2 * qw], ps[:, 0:2 * qw],
                                     AF.Exp, scale=SCALE)
                moff = QCOFF[ci] + k0 * qw
                nc.vector.tensor_mul(pt[:, 0:2 * qw], pt[:, 0:2 * qw],
                                     m01t[:, moff:moff + 2 * qw])
                probs[k0] = (pt, 0)
                probs[k1] = (pt, qw)
            pw = K - KT8 * 128
            ps = ps_sp.tile([128, qw], _F32, tag="sp")
            nc.tensor.matmul(ps[0:pw, 0:qw],
                             kTt[h][:, KT8 * 128:KT8 * 128 + pw],
                             qTt[h][:, qo:qo + qw], start=True, stop=True)
            pt8 = ppool.tile([128, 1024], _BF, tag="p")
            nc.scalar.activation(pt8[0:pw, 0:qw], ps[0:pw, 0:qw],
                                 AF.Exp, scale=SCALE)
            moff = QCOFF[ci] + KT8 * qw
            nc.vector.tensor_mul(pt8[0:pw, 0:qw], pt8[0:pw, 0:qw],
                                 m01t[0:pw, moff:moff + qw])
            probs[KT8] = (pt8, 0)
            return probs

        def pv_den(b, h, probs, qo, qw):
            o_ps = ps_o.tile([128, qw], _F32, tag="o")
            for kt in range(NKT):
                pt, po = probs[kt]
                pw = min(128, K - kt * 128)
                nc.tensor.matmul(
                    o_ps[:, 0:qw], vt[kt][0:pw, h * 128:(h + 1) * 128],
                    pt[0:pw, po:po + qw],
                    start=(kt == 0), stop=(kt == NKT - 1))
            d_ps = ps_d.tile([1, qw], _F32, tag="d")
            zb = (b * NKT) * 15
            for kt in range(NKT):
                pt, po = probs[kt]
                pw = min(128, K - kt * 128)
                zo = zb + kt * 15 + 7
                nc.tensor.matmul(
                    d_ps[:, 0:qw], onespzt[0:pw, zo:zo + 1],
                    pt[0:pw, po:po + qw],
                    start=(kt == 0), stop=(kt == NKT - 1))
            nc.vector.tensor_copy(attn[b][h][:, qo:qo + qw], o_ps[:, 0:qw])
            dst = dstage.tile([1, qw], _F32, tag="ds")
            nc.vector.tensor_copy(dst[:], d_ps[:, 0:qw])
            nc.sync.dma_start(den_b[h:h + 1, qo:qo + qw], dst[:])

        def head(b, h):
            # qc-pipelined: PV of chunk i-1 issues under exp of chunk i
            pend = None
            for ci, (qo, qw) in enumerate(QCH):
                probs = scores_exp_mask(b, h, ci, qo, qw)
                if pend is not None:
                    pv_den(b, h, pend[0], pend[1], pend[2])
                pend = (probs, qo, qw)
            pv_den(b, h, pend[0], pend[1], pend[2])

        def norm(b):
            nc.sync.dma_start(den64[:], den_b[:])
            nc.vector.reciprocal(rec64[:], den64[:])
            nc.sync.dma_start(rec_b[:], rec64[:])
            for h in range(H):
                rt = rpool.tile([1, Q], _F32, tag="rt")
                nc.sync.dma_start(rt[:], rec_b[h:h + 1, :])
                rtb = rbpool.tile([1, Q], _BF, tag="rtb")
                nc.vector.tensor_copy(rtb[:], rt[:])
                bc = bcpool.tile([128, Q], _BF, tag="bc")
                nc.gpsimd.partition_broadcast(bc[:], rtb[:])
                wu = ps_o.tile([128, 512], _F32, tag="o")
                nc.tensor.matmul(wu[:, 0:512], qTt[h][:, 0:128],
                                 bc[:, 0:512], start=True, stop=True)
                nc.vector.tensor_mul(attn[b][h][:], attn[b][h][:], bc[:])
                wu2 = ps_o.tile([128, 512], _F32, tag="o")
                nc.tensor.matmul(wu2[:, :], qTt[h][:, 0:128],
                                 attn[b][h][:, 0:512], start=True, stop=True)

        def oproj_blk(b, wo_t, m):
            for (qo, qw) in QCH:
                ps = ps_pp.tile([128, qw], _F32, tag="pp")
                for kc in range(NM):
                    nc.tensor.matmul(
                        ps[:, 0:qw], wo_t[kc][:, m * 128:(m + 1) * 128],
                        attn[b][kc][:, qo:qo + qw],
                        start=(kc == 0), stop=(kc == NM - 1))
                ot = ostage.tile([128, 512], _F32, tag="os")
                if has_bo:
                    nc.scalar.activation(ot[:, 0:qw], ps[:, 0:qw], AF.Identity,
                                         bias=bot[:, m:m + 1])
                else:
                    nc.scalar.copy(ot[:, 0:qw], ps[:, 0:qw])
                nc.sync.dma_start(outT_d[b, m * 128:(m + 1) * 128, qo:qo + qw],
                                  ot[:, 0:qw])

        # ---- batch 0 projections (serial prologue) -----------------------
        qin0, wq_t, wk_t = [], [], []
        for i in range(NM):
            t = inp.tile([128, Q], _BF, tag="in")
            nc.sync.dma_start(t[:], qinT_d[0, i])
            qin0.append(t)
            wt = wpool.tile([128, D], _BF, tag="w")
            nc.sync.dma_start(wt[:], wq_d[i])
            wq_t.append(wt)
        for i in range(NM):
            wt = wpool.tile([128, D], _BF, tag="w")
            nc.sync.dma_start(wt[:], wk_d[i])
            wk_t.append(wt)
        # mask / denominator operands are first needed in phase B --
        # load them behind the projection operands
        nc.sync.dma_start(m01t[:], m01_d)
        nc.sync.dma_start(onespzt[:], onespz_d)
        for m in range(NM):
            proj_blk(qin0, wq_t, qTt[m], m, bqt if has_bq else None, Q, True)
        kvin0 = load_inp(kvinT_d, 0)
        for m in range(NM):
            proj_blk(kvin0, wk_t, kTt[m], m, bkt if has_bk else None, K, True)
        wv_t = load_w(wv_d)
        for kt in range(NKT):
            vproj_blk(kvin0, wv_t, kt)

        # ---- batch 0 attention; batch 1 qT projection rides along --------
        qin1 = load_inp(qinT_d, 1)
        wq_t1 = load_w(wq_d)
        for h in range(H):
            head(0, h)
            proj_blk(qin1, wq_t1, qTt[h], h, bqt if has_bq else None, Q, True)

        # ---- dense PE tail: batch 1 kT/v projections + batch 0 out-proj --
        kvin1 = load_inp(kvinT_d, 1)
        wk_t1 = load_w(wk_d)
        norm(0)
        for m in range(NM):
            proj_blk(kvin1, wk_t1, kTt[m], m, bkt if has_bk else None, K, True)
        wv_t1 = load_w(wv_d)
        wo_t = load_w(wo_d)
        for kt in range(NKT):
            vproj_blk(kvin1, wv_t1, kt)

        # ---- batch 1 attention; batch 0 out-proj rides along -------------
        for h in range(H):
            head(1, h)
            oproj_blk(0, wo_t, h)

        # ---- batch 1 normalize + out-proj (epilogue) ---------------------
        norm(1)
        wo_t1 = load_w(wo_d)
        for m in range(NM):
            oproj_blk(1, wo_t1, m)

    nc.compile()
    return nc


_prog_cache = {}


def _get_program(key):
    if key not in _prog_cache:
        _prog_cache[key] = _build_program(*key)
    return _prog_cache[key]


def kernel(utterance, lengths, right_context, summary, mems, attention_mask,
           Wq, bq, Wkv, bkv, Wo, bo):
    utterance = np.asarray(utterance, np.float32)
    right_context = np.asarray(right_context, np.float32)
    summary = np.asarray(summary, np.float32)
    mems = np.asarray(mems, np.float32)
    lengths = np.asarray(lengths)
    attention_mask = np.asarray(attention_mask)
    Wq = np.asarray(Wq, np.float32)
    Wkv = np.asarray(Wkv, np.float32)
    Wo = np.asarray(Wo, np.float32)
    bq = np.asarray(bq, np.float32)
    bkv = np.asarray(bkv, np.float32)
    bo = np.asarray(bo, np.float32)

    # ---- host-side prep (layouts, masks) ---------------------------------
    q_in = np.concatenate([right_context, utterance, summary], axis=0)   # (Q,B,D)
    kv_in = np.concatenate([mems, right_context, utterance], axis=0)     # (K,B,D)
    qinT = np.ascontiguousarray(q_in.transpose(2, 1, 0)).astype(BF16)    # (D,B,Q)
    kvinT = np.ascontiguousarray(kv_in.transpose(2, 1, 0))               # (D,B,K) f32

    rcbl = Q - int(lengths.max()) - S
    klengths = (lengths.astype(np.int64) + M + rcbl).astype(np.int64)    # (B,)
    # fold key padding into the data: padded kv columns -> 0 (so v rows are 0)
    gk = np.arange(K)
    for bb in range(B):
        kvinT[:, bb, gk >= klengths[bb]] = 0.0
    kvinT = kvinT.astype(BF16)

    wq_h = np.ascontiguousarray(Wq.T).reshape(NM, 128, D).astype(BF16)
    wk_h = np.ascontiguousarray(Wkv[:D].T).reshape(NM, 128, D).astype(BF16)
    wv_h = np.ascontiguousarray(Wkv[D:].T).reshape(NM, 128, D).astype(BF16)
    wo_h = np.ascontiguousarray(Wo.T).reshape(NM, 128, D).astype(BF16)

    m01 = (~attention_mask).T.astype(BF16)                                # (K,Q)
    m01_p = np.zeros((NKT * 128, Q), BF16)
    m01_p[:K] = m01
    # repack into per-(qchunk, ktile) blocks: col = QCOFF[ci] + kt*qw
    m01r = np.zeros((128, M01RW), BF16)
    for ci, (qo, qw) in enumerate(QCH):
        blk = m01_p[:, qo:qo + qw].reshape(NKT, 128, qw).transpose(1, 0, 2)
        m01r[:, QCOFF[ci]:QCOFF[ci] + NKT * qw] = blk.reshape(128, NKT * qw)

    has_bq = bool(np.any(bq))
    has_bk = bool(np.any(bkv[:D]))
    has_bv = bool(np.any(bkv[D:]))
    has_bo = bool(np.any(bo))

    nc = _get_program((has_bq, has_bk, has_bv, has_bo))

    gidx = np.arange(NKT * 128)
    in_maps = []
    for c in range(NCORES):
        bs = [c * BPC + j for j in range(BPC)]
        onespz = np.zeros((128, BPC * NKT, 15), BF16)
        for j, bb in enumerate(bs):
            col = (gidx < klengths[bb]).astype(BF16).reshape(NKT, 128).T
            onespz[:, j * NKT:(j + 1) * NKT, 7] = col
        onespz = onespz.reshape(128, BPC * NKT * 15)
        im = {
            "qinT": np.ascontiguousarray(
                qinT[:, bs, :].transpose(1, 0, 2).reshape(BPC, NM, 128, Q)),
            "kvinT": np.ascontiguousarray(
                kvinT[:, bs, :].transpose(1, 0, 2).reshape(BPC, NM, 128, K)),
            "wq": wq_h, "wk": wk_h, "wv": wv_h, "wo": wo_h,
            "m01": m01r, "onespz": onespz,
        }
        if has_bq:
            im["bq"] = bq.reshape(NM, 128).T.copy()
        if has_bk:
            im["bk"] = bkv[:D].reshape(NM, 128).T.copy()
        if has_bv:
            im["bvb"] = np.broadcast_to(bkv[D:], (128, D)).copy()
        if has_bo:
            im["bo"] = bo.reshape(NM, 128).T.copy()
        in_maps.append(im)

    res = run_bass_kernel_spmd(nc, in_maps, list(range(NCORES)))

    # ---- gather + unshard -------------------------------------------------
    out = np.empty((Q, B, D), np.float32)
    for c in range(NCORES):
        oT = res.results[c]["outT"]                      # (BPC, D, Q)
        for j in range(BPC):
            out[:, c * BPC + j, :] = oT[j].T
    output = out[:Q - S]                                 # (R+T, B, D)
    out_mems = np.clip(out[Q - S:], -10.0, 10.0)[:-1]    # (S-1, B, D)
    return output, out_mems
